# revision 37
# baseline (speedup 1.0000x reference)
"""DCNv2 (nn_DCNv2_63462436765991) Trainium2 Bass kernel.

Strategy: pure data-parallel over the batch across 8 NeuronCores
(2048 rows/core).  Per core the model runs in 2 passes of 1024 rows.
Activations live in SBUF feature-major as ONE contiguous fp8e4 buffer
per x family ([128 part, 21 k-tiles, 1024 batch]); cross-network and
W0 matmuls run in fp8 with DoubleRow perf mode (2 k-tiles / 256
contraction rows per instruction, 2x PE throughput), fp32 PSUM
accumulation.  Hidden MLP layers and the final matvec stay bf16
(h activations bf16; xfin is cast fp8->bf16 per chunk for the final
layer).  Numerics validated against the fp32 reference: max rel err
~2.9e-3 (gate 2e-2).  Biases fold via an appended ones-row (cross/W0/
final) or the ACT bias port (hidden layers).

Embedding gathers:
  - categorical: dma_gather(transpose=True) over host-padded bf16
    tables ([10000, 128] rows, 256B each; real data in the column half
    matching the feature's destination partition range) into bf16
    staging, then a DVE cast of the 64-row half into the fp8 x buffer.
  - user/item (vocab 100k > int16): indirect_dma_start, one index per
    partition (batch-major staging), then PE transpose + fp8 cast.

x0 row layout (feature-major):  rows 0:64 user, 64:128 item,
128:960 numeric (13 x 64), 960:2624 categorical (26 x 64).
"""

import numpy as np

B = 16384
CORES = 8
B_CORE = B // CORES            # 2048
N_PASS = 2
BC = B_CORE // N_PASS          # 1024 batch per pass
NCH = BC // 512                # matmul N-chunks per pass
EMB = 64
N_NUM = 13
N_CAT = 26
CAT_VOCAB = 10000
D = 2624
KT = 21                        # k-tiles over D (20 x 128 + 64)
MLP = 1024
MT = MLP // 128                # 8
L_CROSS = 4
N_MLP_HID = 3

_CACHE = {}


def _build_nc(n_cross=L_CROSS, with_mlp=True, debug_x0=False, debug_x=False,
              parts=("cat", "num", "uit"), repeats=1):
    import concourse.bass as bass
    import concourse.mybir as mybir
    import concourse.tile as tile
    from concourse import bacc
    from concourse.masks import make_identity

    f32 = mybir.dt.float32
    bf16 = mybir.dt.bfloat16
    f8 = mybir.dt.float8e4
    i32 = mybir.dt.int32
    i16 = mybir.dt.int16
    DR = mybir.MatmulPerfMode.DoubleRow
    MULT = mybir.AluOpType.mult
    ADD = mybir.AluOpType.add
    RELU = mybir.ActivationFunctionType.Relu
    COPY = mybir.ActivationFunctionType.Copy
    SIGM = mybir.ActivationFunctionType.Sigmoid

    # NOTE: num_swdge_queues>1 was tried (parallel gather descgen, −185us
    # device time) but produces nondeterministic wrong gather data on HW
    # even with DMASW-lane-consistent queue assignment — reverted.
    nc = bacc.Bacc("TRN2", target_bir_lowering=False, debug=False,
                   num_swdge_queues=1)
    # SWDGE DMA instructions (gathers / indirects) collected at emission;
    # their hardware queue is assigned post-scheduling from the DMASW lane.
    swdge_insts = []

    # ---- DRAM I/O ----
    u_idx_d = nc.dram_tensor("u_idx", [128, 16], i32, kind="ExternalInput")
    i_idx_d = nc.dram_tensor("i_idx", [128, 16], i32, kind="ExternalInput")
    c_idx_d = nc.dram_tensor("c_idx", [128, N_CAT * 128], i16, kind="ExternalInput")
    numT_d = nc.dram_tensor("numT", [N_NUM + 1, B_CORE], bf16, kind="ExternalInput")
    ndiag_d = nc.dram_tensor("ndiag", [N_NUM + 1, N_NUM * EMB], bf16, kind="ExternalInput")
    uemb_d = nc.dram_tensor("user_emb", [100000, EMB], f32, kind="ExternalInput")
    iemb_d = nc.dram_tensor("item_emb", [100000, EMB], f32, kind="ExternalInput")
    cpad_d = nc.dram_tensor("cat_pad", [N_CAT * CAT_VOCAB, 128], bf16, kind="ExternalInput")
    Wc_d = nc.dram_tensor("Wc", [L_CROSS, D, D], f8, kind="ExternalInput")
    bc_d = nc.dram_tensor("bcx", [L_CROSS, D], f8, kind="ExternalInput")
    W0_d = nc.dram_tensor("W0", [D, MLP], f8, kind="ExternalInput")
    b0_d = nc.dram_tensor("b0", [1, MLP], f8, kind="ExternalInput")
    Wh_d = nc.dram_tensor("Wh", [N_MLP_HID, MLP, MLP], f8, kind="ExternalInput")
    bhT_d = nc.dram_tensor("bhT", [MLP, N_MLP_HID], f32, kind="ExternalInput")
    Wfx_d = nc.dram_tensor("Wfx", [D, 1], f8, kind="ExternalInput")
    Wfh_d = nc.dram_tensor("Wfh", [MLP, 1], bf16, kind="ExternalInput")
    bf_d = nc.dram_tensor("bf", [1, 1], f8, kind="ExternalInput")
    out_d = nc.dram_tensor("out", [1, B_CORE], f32, kind="ExternalOutput")
    if debug_x0:
        x0dbg_d = nc.dram_tensor("x0dbg", [N_PASS, KT, 128, BC], f32, kind="ExternalOutput")
    if debug_x:
        xdbg_d = nc.dram_tensor("xdbg", [N_PASS, KT, 128, BC], f32, kind="ExternalOutput")

    with tile.TileContext(nc) as tc:
        from contextlib import ExitStack
        with ExitStack() as ctx:
            const = ctx.enter_context(tc.tile_pool(name="const", bufs=1))
            xpool = ctx.enter_context(tc.tile_pool(name="xpool", bufs=1))
            wpool = ctx.enter_context(tc.tile_pool(name="wpool", bufs=2))
            stpool = ctx.enter_context(tc.tile_pool(name="stpool", bufs=2))
            tpool = ctx.enter_context(tc.tile_pool(name="tpool", bufs=4))
            bpool = ctx.enter_context(tc.tile_pool(name="bpool", bufs=2))
            zpool = ctx.enter_context(tc.tile_pool(name="zpool", bufs=2))
            mmps = ctx.enter_context(tc.tile_pool(name="mmps", bufs=4, space="PSUM"))
            trps = ctx.enter_context(tc.tile_pool(name="trps", bufs=2, space="PSUM"))

            # ---- per-core constants ----
            uidx = const.tile([128, 16], i32)
            iidx = const.tile([128, 16], i32)
            cidx = const.tile([128, N_CAT * 128], i16)
            numT = const.tile([N_NUM + 1, B_CORE], bf16)
            ndiag = const.tile([N_NUM + 1, N_NUM * EMB], bf16)
            ident = const.tile([128, 128], f32)
            nc.sync.dma_start(uidx[:], u_idx_d[:])
            nc.sync.dma_start(iidx[:], i_idx_d[:])
            nc.sync.dma_start(cidx[:], c_idx_d[:])
            nc.sync.dma_start(numT[:], numT_d[:])
            nc.sync.dma_start(ndiag[:], ndiag_d[:])
            make_identity(nc, ident)

            def alloc_x(prefix):
                # one contiguous fp8 buffer [128, KT, BC]: k-tile t at [:, t, :]
                return xpool.tile([128, KT, BC], f8, tag=prefix, name=prefix)

            def assemble_x0(p, x0):
                # --- user/item: indirect gather issued first so their tile-0
                # data lands early; hardware queue patched post-scheduling ---
                if "uit" in parts:
                    stu = stpool.tile([128, 8, 2, EMB], f32, tag="uit")
                    for c in range(8):
                        pc = p * 8 + c
                        swdge_insts.append(nc.gpsimd.indirect_dma_start(
                            out=stu[:, c, 0, :], out_offset=None, in_=uemb_d[:],
                            in_offset=bass.IndirectOffsetOnAxis(ap=uidx[:, pc:pc + 1], axis=0)))
                        swdge_insts.append(nc.gpsimd.indirect_dma_start(
                            out=stu[:, c, 1, :], out_offset=None, in_=iemb_d[:],
                            in_offset=bass.IndirectOffsetOnAxis(ap=iidx[:, pc:pc + 1], axis=0)))

                # --- categorical gathers (dma_gather transpose): stage bf16,
                # cast the feature's 64-row half into the fp8 x buffer ---
                for f in range(N_CAT if "cat" in parts else 0):
                    trow = 960 + 64 * f
                    t, off = divmod(trow, 128)
                    idx_ap = cidx[:, f * 128 + p * 64: f * 128 + p * 64 + 64]
                    stg = stpool.tile([128, 1, BC], bf16, tag="cstg")
                    swdge_insts.append(nc.gpsimd.dma_gather(
                        out_ap=stg[:], in_ap=cpad_d[f * CAT_VOCAB:(f + 1) * CAT_VOCAB, :],
                        idxs_ap=idx_ap, num_idxs=BC, num_idxs_reg=BC,
                        elem_size=128, transpose=True, single_packet=False))
                    # cast on the Scalar engine: DVE saturates during cross
                    nc.scalar.activation(x0[off:off + 64, t, :], stg[off:off + 64, 0, :], COPY)
                # ones row for the bias fold
                nc.vector.memset(x0[64:65, 20, :], 1.0)

                # --- numeric features: diag-expanded matmul ---
                for m in range(7 if "num" in parts else 0):
                    mw = 128 if m < 6 else 64
                    for ch in range(NCH):
                        ps = mmps.tile([128, 512], mybir.dt.float32, space="PSUM", tag="psacc")
                        nc.tensor.matmul(
                            ps[:mw, :], ndiag[:, m * 128: m * 128 + mw],
                            numT[:, p * BC + ch * 512: p * BC + (ch + 1) * 512],
                            start=True, stop=True)
                        if m < 6:
                            dst = x0[:, 1 + m, ch * 512:(ch + 1) * 512]
                        else:
                            dst = x0[0:64, 7, ch * 512:(ch + 1) * 512]
                        nc.scalar.activation(dst, ps[:mw, :], COPY)

                # --- user/item: PE transpose of the staged rows ---
                if "uit" not in parts:
                    return
                for c in range(8):
                    pst = trps.tile([128, 128], f32, space="PSUM")
                    nc.tensor.transpose(pst[:], stu[:, c, :, :], ident[:])
                    nc.scalar.activation(x0[:, 0, c * 128:(c + 1) * 128], pst[:], COPY)

            def dense_layer(w_src, b_src, xsrc, j, jw, evict):
                """One output j-tile over the 21 fp8 k-tiles of xsrc.

                k-tiles 0..19 run as 10 DoubleRow pair-matmuls (2 k-tiles,
                256 contraction rows per instruction); the 64-row tail plus
                the bias ones-row run as one normal fp8 matmul.
                """
                j0 = j * 128
                wmain = wpool.tile([128, 20, 128], f8, tag="wmain")
                nc.sync.dma_start(
                    wmain[:, :, :jw],
                    w_src[0:2560, j0:j0 + jw]
                    .rearrange("(ko q) n -> q ko n", q=128))
                pss = [mmps.tile([128, 512], mybir.dt.float32, space="PSUM",
                                 name="psacc", tag="psacc") for _ in range(NCH)]
                wlast = wpool.tile([128, 128], f8, tag="wlast")
                nc.sync.dma_start(wlast[0:64, :jw], w_src[2560:2624, j0:j0 + jw])
                nc.sync.dma_start(wlast[64:65, :jw], b_src[0:1, j0:j0 + jw])
                for t in range(10):
                    for ch in range(NCH):
                        nc.tensor.matmul(
                            pss[ch][:jw, :], wmain[:, 2 * t:2 * t + 2, :jw],
                            xsrc[:, 2 * t:2 * t + 2, ch * 512:(ch + 1) * 512],
                            start=(t == 0), stop=False, perf_mode=DR)
                for ch in range(NCH):
                    nc.tensor.matmul(
                        pss[ch][:jw, :], wlast[0:65, :jw],
                        xsrc[0:65, 20, ch * 512:(ch + 1) * 512],
                        start=False, stop=True)
                for ch in range(NCH):
                    evict(pss[ch], ch)

            def cross_layer(i, x0, xsrc, xdst):
                for j in range(KT):
                    jw = 128 if j < 20 else 64

                    def evict(ps, ch, j=j, jw=jw):
                        sl = slice(ch * 512, (ch + 1) * 512)
                        tmp = tpool.tile([128, 512], bf16, tag="evt")
                        nc.vector.tensor_tensor(tmp[:jw, :], ps[:jw, :], x0[:jw, j, sl], MULT)
                        nc.vector.tensor_tensor(xdst[:jw, j, sl], tmp[:jw, :], xsrc[:jw, j, sl], ADD)

                    dense_layer(Wc_d[i], bc_d[i:i + 1], xsrc, j, jw, evict)

            def mlp_w0(x0, ha, ha8):
                for j in range(MT):
                    def evict(ps, ch, j=j):
                        sl = slice(ch * 512, (ch + 1) * 512)
                        nc.scalar.activation(ha[j][:, sl], ps[:, :], RELU)
                        nc.vector.tensor_copy(ha8[:, j, sl], ha[j][:, sl])
                    dense_layer(W0_d[:], b0_d[:], x0, j, 128, evict)

            def mlp_hidden(l, src8, dst, dst8):
                for j in range(MT):
                    j0 = j * 128
                    whm = wpool.tile([128, MT, 128], f8, tag="whid")
                    nc.sync.dma_start(
                        whm[:, :, :], Wh_d[l, :, j0:j0 + 128]
                        .rearrange("(ko q) n -> q ko n", q=128))
                    bias = bpool.tile([128, 1], f32, tag="bias")
                    nc.sync.dma_start(bias[:], bhT_d[j0:j0 + 128, l:l + 1])
                    pss = [mmps.tile([128, 512], mybir.dt.float32, space="PSUM",
                                     name="psacc", tag="psacc") for _ in range(NCH)]
                    for t in range(MT // 2):
                        for ch in range(NCH):
                            nc.tensor.matmul(
                                pss[ch][:, :], whm[:, 2 * t:2 * t + 2, :],
                                src8[:, 2 * t:2 * t + 2, ch * 512:(ch + 1) * 512],
                                start=(t == 0), stop=(t == MT // 2 - 1),
                                perf_mode=DR)
                    for ch in range(NCH):
                        sl = slice(ch * 512, (ch + 1) * 512)
                        nc.scalar.activation(dst[j][:, sl], pss[ch][:, :],
                                             RELU, bias=bias[:])
                        if dst8 is not None:
                            nc.vector.tensor_copy(dst8[:, j, sl], dst[j][:, sl])

            def final_layer(p, xfin, hfin):
                wfm = wpool.tile([128, 20, 1], f8, tag="wfm")
                nc.sync.dma_start(
                    wfm[:], Wfx_d[0:2560, 0:1].rearrange("(ko q) n -> q ko n", q=128))
                wfl = wpool.tile([128, 1], f8, tag="wfl")
                nc.sync.dma_start(wfl[0:64, :], Wfx_d[2560:2624, 0:1])
                nc.sync.dma_start(wfl[64:65, :], bf_d[:])
                wfh = wpool.tile([128, MT, 1], bf16, tag="wfh")
                nc.sync.dma_start(
                    wfh[:], Wfh_d[:, 0:1].rearrange("(ko q) n -> q ko n", q=128))
                for ch in range(NCH):
                    sl = slice(ch * 512, (ch + 1) * 512)
                    zps = mmps.tile([128, 512], mybir.dt.float32, space="PSUM", tag="psacc")
                    for t in range(20):
                        nc.tensor.matmul(zps[0:1, :], wfm[:, t, :], xfin[:, t, sl],
                                         start=(t == 0), stop=False)
                    nc.tensor.matmul(zps[0:1, :], wfl[0:65, :], xfin[0:65, 20, sl],
                                     start=False, stop=False)
                    for t in range(MT):
                        nc.tensor.matmul(zps[0:1, :], wfh[:, t, :],
                                         hfin[t][:, sl],
                                         start=False, stop=(t == MT - 1))
                    zsb = zpool.tile([1, 512], f32, tag="zsb")
                    nc.scalar.activation(zsb[:], zps[0:1, :], SIGM)
                    nc.sync.dma_start(
                        out_d[0:1, p * BC + ch * 512: p * BC + (ch + 1) * 512], zsb[:])

            for pi, p in enumerate([pp for _ in range(repeats) for pp in range(N_PASS)]):
                # Pass p's x0 gets its own family (0 or 3) so pass p+1's
                # gathers start immediately instead of WAR-waiting on the
                # cross ping-pong buffers; ping-pong uses families 1/2.
                x0fam = 0 if pi % 2 == 0 else 3
                afam = 1
                x0 = alloc_x(f"xs{x0fam}_")
                assemble_x0(p, x0)
                if debug_x0:
                    for t in range(KT):
                        dbg = tpool.tile([128, 512], f32, tag="dbg")
                        for ch in range(NCH):
                            nc.vector.tensor_copy(dbg[:], x0[:, t, ch * 512:(ch + 1) * 512])
                            nc.sync.dma_start(
                                x0dbg_d[p, t, :, ch * 512:(ch + 1) * 512], dbg[:])
                bufs = [alloc_x(f"xs{afam}_"), alloc_x("xs2_")]
                for bb_ in bufs:
                    nc.vector.memset(bb_[64:65, 20, :], 1.0)
                xsrc = x0
                for i in range(n_cross):
                    xdst = bufs[i % 2]
                    cross_layer(i, x0, xsrc, xdst)
                    xsrc = xdst
                xfin = xsrc
                if debug_x:
                    for t in range(KT):
                        dbg = tpool.tile([128, 512], f32, tag="dbg")
                        for ch in range(NCH):
                            nc.vector.tensor_copy(dbg[:], xfin[:, t, ch * 512:(ch + 1) * 512])
                            nc.sync.dma_start(
                                xdbg_d[p, t, :, ch * 512:(ch + 1) * 512], dbg[:])
                if with_mlp:
                    ha = [xpool.tile([128, BC], bf16, tag=f"ha{t}", name=f"ha{t}") for t in range(MT)]
                    hb = [xpool.tile([128, BC], bf16, tag=f"hb{t}", name=f"hb{t}") for t in range(MT)]
                    ha8 = xpool.tile([128, MT, BC], f8, tag="ha8", name="ha8")
                    hb8 = xpool.tile([128, MT, BC], f8, tag="hb8", name="hb8")
                    mlp_w0(x0, ha, ha8)
                    hsrc, hsrc8 = ha, ha8
                    for l in range(N_MLP_HID):
                        hdst = hb if l % 2 == 0 else ha
                        hdst8 = hb8 if l % 2 == 0 else ha8
                        last = l == N_MLP_HID - 1
                        mlp_hidden(l, hsrc8, hdst, None if last else hdst8)
                        hsrc, hsrc8 = hdst, hdst8
                    final_layer(p, xfin, hsrc)

    # Route each SWDGE DMA to the hardware queue matching its DMASW lane.
    # Tile's sem assignment distributes SWDGE DMAs round-robin over 8 DMASW
    # lanes (one vector-clock dim + sem per lane) and assumes completions
    # within a lane are FIFO.  Mapping queue = lane % 4 keeps every lane on
    # a single hardware queue, so in-lane FIFO still holds while the 4
    # queues run descriptor generation in parallel.
    if nc.num_swdge_queues > 1:
        from concourse.tile_scheduler import PROC_NAME_TO_IDX
        sw0 = PROC_NAME_TO_IDX["DMASW0"]
        sw7 = PROC_NAME_TO_IDX["DMASW7"]
        for inst in swdge_insts:
            if hasattr(inst, "ins"):
                inst = inst.ins
            proc = inst.bass_scheduled_proc
            assert proc is not None and sw0 <= proc <= sw7, (inst.name, proc)
            q = (proc - sw0) % nc.num_swdge_queues
            if isinstance(inst, mybir.InstDMAGatherAnt):
                inst.queue_num = q
            elif isinstance(inst, mybir.InstDMACopy):
                inst.queue = f"qPoolDynamic{q if q else ''}"
            else:
                raise AssertionError(f"unexpected SWDGE inst {type(inst)}")

    nc.compile()
    return nc


# needed at module level for the builder
import concourse.bass as bass  # noqa: E402


def _prep_core_inputs(core, user_input, item_input, numeric_feats, categorical_feats,
                      shared):
    r0 = core * B_CORE
    u = user_input[r0:r0 + B_CORE]
    it = item_input[r0:r0 + B_CORE]
    num = numeric_feats[r0:r0 + B_CORE]
    cat = categorical_feats[r0:r0 + B_CORE]

    u_idx = np.ascontiguousarray(u.reshape(16, 128).T).astype(np.int32)
    i_idx = np.ascontiguousarray(it.reshape(16, 128).T).astype(np.int32)

    c_idx = np.zeros((128, N_CAT * 128), np.int16)
    for f in range(N_CAT):
        for p in range(N_PASS):
            seg = cat[p * BC:(p + 1) * BC, f].astype(np.int16)
            blk = seg.reshape(BC // 16, 16).T          # wrap-A: idx i at [i%16, i//16]
            c_idx[:, f * 128 + p * 64: f * 128 + (p + 1) * 64] = np.tile(blk, (8, 1))

    import ml_dtypes
    numT = np.empty((N_NUM + 1, B_CORE), ml_dtypes.bfloat16)
    numT[:N_NUM] = num.T.astype(ml_dtypes.bfloat16)
    numT[N_NUM] = 1.0

    return {
        "u_idx": u_idx, "i_idx": i_idx, "c_idx": c_idx, "numT": numT,
        **shared,
    }


def _prep_shared(num_W, num_b, user_emb, item_emb, cat_tables,
                 Wc, bc, W0, b0, Wh, bh, Wf, bf):
    import ml_dtypes
    ndiag = np.zeros((N_NUM + 1, N_NUM * EMB), np.float32)
    for f in range(N_NUM):
        ndiag[f, f * EMB:(f + 1) * EMB] = num_W[f]
    ndiag[N_NUM] = num_b.reshape(-1)

    cat_pad = np.zeros((N_CAT * CAT_VOCAB, 128), ml_dtypes.bfloat16)
    ct = cat_tables.astype(ml_dtypes.bfloat16)
    for f in range(N_CAT):
        sl = slice(f * CAT_VOCAB, (f + 1) * CAT_VOCAB)
        if f % 2 == 0:   # destination rows 64:128 of the x^T tile
            cat_pad[sl, 64:128] = ct[f]
        else:            # destination rows 0:64
            cat_pad[sl, 0:64] = ct[f]

    bf16 = ml_dtypes.bfloat16
    fp8 = ml_dtypes.float8_e4m3
    return {
        "ndiag": ndiag.astype(bf16),
        "user_emb": np.ascontiguousarray(user_emb, np.float32),
        "item_emb": np.ascontiguousarray(item_emb, np.float32),
        "cat_pad": cat_pad,
        "Wc": np.ascontiguousarray(Wc, np.float32).astype(fp8),
        "bcx": np.ascontiguousarray(bc, np.float32).astype(fp8),
        "W0": np.ascontiguousarray(W0, np.float32).astype(fp8),
        "b0": np.ascontiguousarray(b0, np.float32).reshape(1, MLP).astype(fp8),
        "Wh": np.ascontiguousarray(Wh, np.float32).astype(fp8),
        "bhT": np.ascontiguousarray(np.asarray(bh, np.float32).T),
        "Wfx": np.ascontiguousarray(Wf[:D], np.float32).astype(fp8),
        "Wfh": np.ascontiguousarray(Wf[D:], np.float32).astype(bf16),
        "bf": np.asarray(bf, np.float32).reshape(1, 1).astype(fp8),
    }


def make_in_maps(user_input, item_input, numeric_feats, categorical_feats,
                 user_emb, item_emb, cat_tables, num_W, num_b,
                 Wc, bc, W0, b0, Wh, bh, Wf, bf):
    user_input = np.asarray(user_input).astype(np.int64)
    item_input = np.asarray(item_input).astype(np.int64)
    numeric_feats = np.asarray(numeric_feats, np.float32)
    categorical_feats = np.asarray(categorical_feats).astype(np.int64)
    shared = _prep_shared(np.asarray(num_W, np.float32), np.asarray(num_b, np.float32),
                          np.asarray(user_emb), np.asarray(item_emb),
                          np.asarray(cat_tables, np.float32),
                          np.asarray(Wc), np.asarray(bc), np.asarray(W0),
                          np.asarray(b0), np.asarray(Wh), np.asarray(bh),
                          np.asarray(Wf), np.asarray(bf))
    return [
        _prep_core_inputs(core, user_input, item_input, numeric_feats,
                          categorical_feats, shared)
        for core in range(CORES)
    ]


def get_nc(**flags):
    key = tuple(sorted(flags.items()))
    if key not in _CACHE:
        _CACHE[key] = _build_nc(**flags)
    return _CACHE[key]


def kernel(**inputs) -> np.ndarray:
    from concourse.bass_utils import run_bass_kernel_spmd
    nc = get_nc()
    in_maps = make_in_maps(**inputs)
    res = run_bass_kernel_spmd(nc, in_maps, list(range(CORES)))
    out = np.concatenate([res.results[i]["out"][0] for i in range(CORES)])
    return out.reshape(B, 1).astype(np.float32)



# revision 44
# speedup vs baseline: 1.4230x; 1.4230x over previous
"""DCNv2 (nn_DCNv2_63462436765991) Trainium2 Bass kernel.

Strategy: pure data-parallel over the batch across 8 NeuronCores
(2048 rows/core).  Per core the model runs in 2 passes of 1024 rows.
Activations live in SBUF feature-major as ONE contiguous fp8e4 buffer
per x family ([128 part, 21 k-tiles, 1024 batch]); cross-network and
W0 matmuls run in fp8 with DoubleRow perf mode (2 k-tiles / 256
contraction rows per instruction, 2x PE throughput), fp32 PSUM
accumulation.  Hidden MLP layers and the final matvec stay bf16
(h activations bf16; xfin is cast fp8->bf16 per chunk for the final
layer).  Numerics validated against the fp32 reference: max rel err
~2.9e-3 (gate 2e-2).  Biases fold via an appended ones-row (cross/W0/
final) or the ACT bias port (hidden layers).

Embedding gathers:
  - categorical: dma_gather(transpose=True) over host-padded bf16
    tables ([10000, 128] rows, 256B each; real data in the column half
    matching the feature's destination partition range) into bf16
    staging, then a DVE cast of the 64-row half into the fp8 x buffer.
  - user/item (vocab 100k > int16): indirect_dma_start, one index per
    partition (batch-major staging), then PE transpose + fp8 cast.

x0 row layout (feature-major):  rows 0:64 user, 64:128 item,
128:960 numeric (13 x 64), 960:2624 categorical (26 x 64).
"""

import numpy as np

B = 16384
CORES = 8
B_CORE = B // CORES            # 2048
N_PASS = 2
BC = B_CORE // N_PASS          # 1024 batch per pass
NCH = BC // 512                # matmul N-chunks per pass
EMB = 64
N_NUM = 13
N_CAT = 26
CAT_VOCAB = 10000
D = 2624
KT = 21                        # k-tiles over D (20 x 128 + 64)
MLP = 1024
MT = MLP // 128                # 8
L_CROSS = 4
N_MLP_HID = 3

_CACHE = {}


def _build_nc(n_cross=L_CROSS, with_mlp=True, debug_x0=False, debug_x=False,
              parts=("cat", "num", "uit"), repeats=1):
    import concourse.bass as bass
    import concourse.mybir as mybir
    import concourse.tile as tile
    from concourse import bacc
    from concourse.masks import make_identity

    f32 = mybir.dt.float32
    bf16 = mybir.dt.bfloat16
    f8 = mybir.dt.float8e4
    i32 = mybir.dt.int32
    i16 = mybir.dt.int16
    DR = mybir.MatmulPerfMode.DoubleRow
    MULT = mybir.AluOpType.mult
    ADD = mybir.AluOpType.add
    RELU = mybir.ActivationFunctionType.Relu
    COPY = mybir.ActivationFunctionType.Copy
    SIGM = mybir.ActivationFunctionType.Sigmoid

    # NOTE: num_swdge_queues>1 was tried (parallel gather descgen, −185us
    # device time) but produces nondeterministic wrong gather data on HW
    # even with DMASW-lane-consistent queue assignment — reverted.
    nc = bacc.Bacc("TRN2", target_bir_lowering=False, debug=False,
                   num_swdge_queues=1)
    # SWDGE DMA instructions (gathers / indirects) collected at emission;
    # their hardware queue is assigned post-scheduling from the DMASW lane.
    swdge_insts = []

    # ---- DRAM I/O ----
    u_idx_d = nc.dram_tensor("u_idx", [128, 16], i32, kind="ExternalInput")
    i_idx_d = nc.dram_tensor("i_idx", [128, 16], i32, kind="ExternalInput")
    c_idx_d = nc.dram_tensor("c_idx", [128, N_CAT * 128], i16, kind="ExternalInput")
    numT_d = nc.dram_tensor("numT", [N_NUM + 1, B_CORE], bf16, kind="ExternalInput")
    ndiag_d = nc.dram_tensor("ndiag", [N_NUM + 1, N_NUM * EMB], bf16, kind="ExternalInput")
    uemb_d = nc.dram_tensor("user_emb", [100000, EMB], f32, kind="ExternalInput")
    iemb_d = nc.dram_tensor("item_emb", [100000, EMB], f32, kind="ExternalInput")
    cpad_d = nc.dram_tensor("cat_pad", [N_CAT * CAT_VOCAB, 128], bf16, kind="ExternalInput")
    Wc_d = nc.dram_tensor("Wc", [L_CROSS, D, D], f8, kind="ExternalInput")
    bc_d = nc.dram_tensor("bcx", [L_CROSS, D], f8, kind="ExternalInput")
    W0_d = nc.dram_tensor("W0", [D, MLP], f8, kind="ExternalInput")
    b0_d = nc.dram_tensor("b0", [1, MLP], f8, kind="ExternalInput")
    Wh_d = nc.dram_tensor("Wh", [N_MLP_HID, MLP, MLP], f8, kind="ExternalInput")
    bhT_d = nc.dram_tensor("bhT", [MLP, N_MLP_HID], f32, kind="ExternalInput")
    Wfx_d = nc.dram_tensor("Wfx", [D, 1], f8, kind="ExternalInput")
    Wfh_d = nc.dram_tensor("Wfh", [MLP, 1], bf16, kind="ExternalInput")
    bf_d = nc.dram_tensor("bf", [1, 1], f8, kind="ExternalInput")
    out_d = nc.dram_tensor("out", [1, B_CORE], f32, kind="ExternalOutput")
    if debug_x0:
        x0dbg_d = nc.dram_tensor("x0dbg", [N_PASS, KT, 128, BC], f32, kind="ExternalOutput")
    if debug_x:
        xdbg_d = nc.dram_tensor("xdbg", [N_PASS, KT, 128, BC], f32, kind="ExternalOutput")

    with tile.TileContext(nc) as tc:
        from contextlib import ExitStack
        with ExitStack() as ctx:
            const = ctx.enter_context(tc.tile_pool(name="const", bufs=1))
            xpool = ctx.enter_context(tc.tile_pool(name="xpool", bufs=1))
            wpool = ctx.enter_context(tc.tile_pool(name="wpool", bufs=2))
            stpool = ctx.enter_context(tc.tile_pool(name="stpool", bufs=2))
            tpool = ctx.enter_context(tc.tile_pool(name="tpool", bufs=4))
            bpool = ctx.enter_context(tc.tile_pool(name="bpool", bufs=2))
            zpool = ctx.enter_context(tc.tile_pool(name="zpool", bufs=1))
            mmps = ctx.enter_context(tc.tile_pool(name="mmps", bufs=4, space="PSUM"))
            trps = ctx.enter_context(tc.tile_pool(name="trps", bufs=2, space="PSUM"))

            # ---- per-core constants ----
            uidx = const.tile([128, 16], i32)
            iidx = const.tile([128, 16], i32)
            cidx = const.tile([128, N_CAT * 128], i16)
            numT = const.tile([N_NUM + 1, B_CORE], bf16)
            ndiag = const.tile([N_NUM + 1, N_NUM * EMB], bf16)
            ident = const.tile([128, 128], f32)
            nc.sync.dma_start(uidx[:], u_idx_d[:])
            nc.sync.dma_start(iidx[:], i_idx_d[:])
            nc.sync.dma_start(cidx[:], c_idx_d[:])
            nc.sync.dma_start(numT[:], numT_d[:])
            nc.sync.dma_start(ndiag[:], ndiag_d[:])
            make_identity(nc, ident)

            def alloc_x(prefix):
                # one contiguous fp8 buffer [128, KT, BC]: k-tile t at [:, t, :]
                return xpool.tile([128, KT, BC], f8, tag=prefix, name=prefix)

            def assemble_x0(p, x0):
                # --- user/item: indirect gather issued first so their tile-0
                # data lands early; hardware queue patched post-scheduling ---
                if "uit" in parts:
                    stu = stpool.tile([128, 8, 2, EMB], f32, tag="uit")
                    for c in range(8):
                        pc = p * 8 + c
                        swdge_insts.append(nc.gpsimd.indirect_dma_start(
                            out=stu[:, c, 0, :], out_offset=None, in_=uemb_d[:],
                            in_offset=bass.IndirectOffsetOnAxis(ap=uidx[:, pc:pc + 1], axis=0)))
                        swdge_insts.append(nc.gpsimd.indirect_dma_start(
                            out=stu[:, c, 1, :], out_offset=None, in_=iemb_d[:],
                            in_offset=bass.IndirectOffsetOnAxis(ap=iidx[:, pc:pc + 1], axis=0)))

                # --- categorical gathers (dma_gather transpose): stage bf16,
                # cast the feature's 64-row half into the fp8 x buffer ---
                for f in range(N_CAT if "cat" in parts else 0):
                    trow = 960 + 64 * f
                    t, off = divmod(trow, 128)
                    idx_ap = cidx[:, f * 128 + p * 64: f * 128 + p * 64 + 64]
                    stg = stpool.tile([128, 1, BC], bf16, tag="cstg")
                    swdge_insts.append(nc.gpsimd.dma_gather(
                        out_ap=stg[:], in_ap=cpad_d[f * CAT_VOCAB:(f + 1) * CAT_VOCAB, :],
                        idxs_ap=idx_ap, num_idxs=BC, num_idxs_reg=BC,
                        elem_size=128, transpose=True, single_packet=False))
                    # cast on the Scalar engine: DVE saturates during cross
                    nc.scalar.activation(x0[off:off + 64, t, :], stg[off:off + 64, 0, :], COPY)
                # ones row for the bias fold
                nc.vector.memset(x0[64:65, 20, :], 1.0)

                # --- numeric features: diag-expanded matmul ---
                for m in range(7 if "num" in parts else 0):
                    mw = 128 if m < 6 else 64
                    for ch in range(NCH):
                        ps = mmps.tile([128, 512], mybir.dt.float32, space="PSUM", tag="psacc")
                        nc.tensor.matmul(
                            ps[:mw, :], ndiag[:, m * 128: m * 128 + mw],
                            numT[:, p * BC + ch * 512: p * BC + (ch + 1) * 512],
                            start=True, stop=True)
                        if m < 6:
                            dst = x0[:, 1 + m, ch * 512:(ch + 1) * 512]
                        else:
                            dst = x0[0:64, 7, ch * 512:(ch + 1) * 512]
                        nc.scalar.activation(dst, ps[:mw, :], COPY)

                # --- user/item: PE transpose of the staged rows ---
                if "uit" not in parts:
                    return
                for c in range(8):
                    pst = trps.tile([128, 128], f32, space="PSUM")
                    nc.tensor.transpose(pst[:], stu[:, c, :, :], ident[:])
                    nc.scalar.activation(x0[:, 0, c * 128:(c + 1) * 128], pst[:], COPY)

            def dense_layer(w_src, b_src, xsrc, j, jw, evict):
                """One output j-tile over the 21 fp8 k-tiles of xsrc.

                k-tiles 0..19 run as 10 DoubleRow pair-matmuls (2 k-tiles,
                256 contraction rows per instruction); the 64-row tail plus
                the bias ones-row run as one normal fp8 matmul.
                """
                j0 = j * 128
                wmain = wpool.tile([128, 20, 128], f8, tag="wmain")
                nc.sync.dma_start(
                    wmain[:, :, :jw],
                    w_src[0:2560, j0:j0 + jw]
                    .rearrange("(ko q) n -> q ko n", q=128))
                pss = [mmps.tile([128, 512], mybir.dt.float32, space="PSUM",
                                 name="psacc", tag="psacc") for _ in range(NCH)]
                wlast = wpool.tile([128, 128], f8, tag="wlast")
                nc.sync.dma_start(wlast[0:64, :jw], w_src[2560:2624, j0:j0 + jw])
                nc.sync.dma_start(wlast[64:65, :jw], b_src[0:1, j0:j0 + jw])
                for t in range(10):
                    for ch in range(NCH):
                        nc.tensor.matmul(
                            pss[ch][:jw, :], wmain[:, 2 * t:2 * t + 2, :jw],
                            xsrc[:, 2 * t:2 * t + 2, ch * 512:(ch + 1) * 512],
                            start=(t == 0), stop=False, perf_mode=DR)
                for ch in range(NCH):
                    nc.tensor.matmul(
                        pss[ch][:jw, :], wlast[0:65, :jw],
                        xsrc[0:65, 20, ch * 512:(ch + 1) * 512],
                        start=False, stop=True)
                for ch in range(NCH):
                    evict(pss[ch], ch)

            def cross_layer(i, x0, xsrc, xdst):
                for j in range(KT):
                    jw = 128 if j < 20 else 64

                    def evict(ps, ch, j=j, jw=jw):
                        sl = slice(ch * 512, (ch + 1) * 512)
                        tmp = tpool.tile([128, 512], bf16, tag="evt")
                        nc.vector.tensor_tensor(tmp[:jw, :], ps[:jw, :], x0[:jw, j, sl], MULT)
                        nc.vector.tensor_tensor(xdst[:jw, j, sl], tmp[:jw, :], xsrc[:jw, j, sl], ADD)

                    dense_layer(Wc_d[i], bc_d[i:i + 1], xsrc, j, jw, evict)

            S1 = 7   # k-pairs (tiles 0..13) computed in split stage 1

            def cross_layer0_split(x0, xdst):
                """Layer 0 with split-k: stage 1 runs on the early-arriving
                k-tiles for every j while the later categorical gathers are
                still streaming in (fills the pass-0 assembly stall); stage 2
                finishes k and fuses the staged partial during eviction."""
                J1 = 16   # j-tiles with split-k; the rest run full-k at the end
                ypart = xpool.tile([128, J1, BC], bf16, tag="ypart", name="ypart")
                for j in range(J1):
                    jw = 128
                    j0 = j * 128
                    wmain = wpool.tile([128, S1 * 2, 128], f8, tag="wm1")
                    nc.sync.dma_start(
                        wmain[:, :, :jw],
                        Wc_d[0][0:S1 * 256, j0:j0 + jw]
                        .rearrange("(ko q) n -> q ko n", q=128))
                    pss = [mmps.tile([128, 512], mybir.dt.float32, space="PSUM",
                                     name="psacc", tag="psacc") for _ in range(NCH)]
                    for t in range(S1):
                        for ch in range(NCH):
                            nc.tensor.matmul(
                                pss[ch][:jw, :], wmain[:, 2 * t:2 * t + 2, :jw],
                                x0[:, 2 * t:2 * t + 2, ch * 512:(ch + 1) * 512],
                                start=(t == 0), stop=(t == S1 - 1), perf_mode=DR)
                    for ch in range(NCH):
                        nc.scalar.activation(
                            ypart[:jw, j, ch * 512:(ch + 1) * 512], pss[ch][:jw, :], COPY)
                for j in range(J1):
                    jw = 128
                    j0 = j * 128
                    wm2 = wpool.tile([128, (10 - S1) * 2, 128], f8, tag="wm2")
                    nc.sync.dma_start(
                        wm2[:, :, :jw],
                        Wc_d[0][S1 * 256:2560, j0:j0 + jw]
                        .rearrange("(ko q) n -> q ko n", q=128))
                    wlast = wpool.tile([128, 128], f8, tag="wlast")
                    nc.sync.dma_start(wlast[0:64, :jw], Wc_d[0][2560:2624, j0:j0 + jw])
                    nc.sync.dma_start(wlast[64:65, :jw], bc_d[0:1, j0:j0 + jw])
                    pss = [mmps.tile([128, 512], mybir.dt.float32, space="PSUM",
                                     name="psacc", tag="psacc") for _ in range(NCH)]
                    for t in range(10 - S1):
                        for ch in range(NCH):
                            nc.tensor.matmul(
                                pss[ch][:jw, :], wm2[:, 2 * t:2 * t + 2, :jw],
                                x0[:, 2 * (S1 + t):2 * (S1 + t) + 2, ch * 512:(ch + 1) * 512],
                                start=(t == 0), stop=False, perf_mode=DR)
                    for ch in range(NCH):
                        nc.tensor.matmul(
                            pss[ch][:jw, :], wlast[0:65, :jw],
                            x0[0:65, 20, ch * 512:(ch + 1) * 512],
                            start=False, stop=True)
                    for ch in range(NCH):
                        sl = slice(ch * 512, (ch + 1) * 512)
                        tmp = tpool.tile([128, 512], bf16, tag="evt")
                        nc.vector.tensor_tensor(tmp[:jw, :], pss[ch][:jw, :], ypart[:jw, j, sl], ADD)
                        tmp2 = tpool.tile([128, 512], bf16, tag="evt")
                        nc.vector.tensor_tensor(tmp2[:jw, :], tmp[:jw, :], x0[:jw, j, sl], MULT)
                        nc.vector.tensor_tensor(xdst[:jw, j, sl], tmp2[:jw, :], x0[:jw, j, sl], ADD)
                for j in range(J1, KT):
                    jw = 128 if j < 20 else 64

                    def evict(ps, ch, j=j, jw=jw):
                        sl = slice(ch * 512, (ch + 1) * 512)
                        tmp = tpool.tile([128, 512], bf16, tag="evt")
                        nc.vector.tensor_tensor(tmp[:jw, :], ps[:jw, :], x0[:jw, j, sl], MULT)
                        nc.vector.tensor_tensor(xdst[:jw, j, sl], tmp[:jw, :], x0[:jw, j, sl], ADD)

                    dense_layer(Wc_d[0], bc_d[0:1], x0, j, jw, evict)

            def mlp_w0(x0, ha, ha8):
                for j in range(MT):
                    def evict(ps, ch, j=j):
                        sl = slice(ch * 512, (ch + 1) * 512)
                        nc.scalar.activation(ha[j][:, sl], ps[:, :], RELU)
                        nc.vector.tensor_copy(ha8[:, j, sl], ha[j][:, sl])
                    dense_layer(W0_d[:], b0_d[:], x0, j, 128, evict)

            def mlp_hidden(l, src8, dst, dst8):
                for j in range(MT):
                    j0 = j * 128
                    whm = wpool.tile([128, MT, 128], f8, tag="whid")
                    nc.sync.dma_start(
                        whm[:, :, :], Wh_d[l, :, j0:j0 + 128]
                        .rearrange("(ko q) n -> q ko n", q=128))
                    bias = bpool.tile([128, 1], f32, tag="bias")
                    nc.sync.dma_start(bias[:], bhT_d[j0:j0 + 128, l:l + 1])
                    pss = [mmps.tile([128, 512], mybir.dt.float32, space="PSUM",
                                     name="psacc", tag="psacc") for _ in range(NCH)]
                    for t in range(MT // 2):
                        for ch in range(NCH):
                            nc.tensor.matmul(
                                pss[ch][:, :], whm[:, 2 * t:2 * t + 2, :],
                                src8[:, 2 * t:2 * t + 2, ch * 512:(ch + 1) * 512],
                                start=(t == 0), stop=(t == MT // 2 - 1),
                                perf_mode=DR)
                    for ch in range(NCH):
                        sl = slice(ch * 512, (ch + 1) * 512)
                        nc.scalar.activation(dst[j][:, sl], pss[ch][:, :],
                                             RELU, bias=bias[:])
                        if dst8 is not None:
                            nc.vector.tensor_copy(dst8[:, j, sl], dst[j][:, sl])

            def final_layer(p, xfin, hfin):
                wfm = wpool.tile([128, 20, 1], f8, tag="wfm")
                nc.sync.dma_start(
                    wfm[:], Wfx_d[0:2560, 0:1].rearrange("(ko q) n -> q ko n", q=128))
                wfl = wpool.tile([128, 1], f8, tag="wfl")
                nc.sync.dma_start(wfl[0:64, :], Wfx_d[2560:2624, 0:1])
                nc.sync.dma_start(wfl[64:65, :], bf_d[:])
                wfh = wpool.tile([128, MT, 1], bf16, tag="wfh")
                nc.sync.dma_start(
                    wfh[:], Wfh_d[:, 0:1].rearrange("(ko q) n -> q ko n", q=128))
                for ch in range(NCH):
                    sl = slice(ch * 512, (ch + 1) * 512)
                    zps = mmps.tile([128, 512], mybir.dt.float32, space="PSUM", tag="psacc")
                    for t in range(20):
                        nc.tensor.matmul(zps[0:1, :], wfm[:, t, :], xfin[:, t, sl],
                                         start=(t == 0), stop=False)
                    nc.tensor.matmul(zps[0:1, :], wfl[0:65, :], xfin[0:65, 20, sl],
                                     start=False, stop=False)
                    for t in range(MT):
                        nc.tensor.matmul(zps[0:1, :], wfh[:, t, :],
                                         hfin[t][:, sl],
                                         start=False, stop=(t == MT - 1))
                    zsb = zpool.tile([1, 512], f32, tag="zsb")
                    nc.scalar.activation(zsb[:], zps[0:1, :], SIGM)
                    nc.sync.dma_start(
                        out_d[0:1, p * BC + ch * 512: p * BC + (ch + 1) * 512], zsb[:])

            for pi, p in enumerate([pp for _ in range(repeats) for pp in range(N_PASS)]):
                # Pass p's x0 gets its own family (0 or 3) so pass p+1's
                # gathers start immediately instead of WAR-waiting on the
                # cross ping-pong buffers; ping-pong uses families 1/2.
                x0fam = 0 if pi % 2 == 0 else 3
                afam = 1
                x0 = alloc_x(f"xs{x0fam}_")
                assemble_x0(p, x0)
                if debug_x0:
                    for t in range(KT):
                        dbg = tpool.tile([128, 512], f32, tag="dbg")
                        for ch in range(NCH):
                            nc.vector.tensor_copy(dbg[:], x0[:, t, ch * 512:(ch + 1) * 512])
                            nc.sync.dma_start(
                                x0dbg_d[p, t, :, ch * 512:(ch + 1) * 512], dbg[:])
                bufs = [alloc_x(f"xs{afam}_"), alloc_x("xs2_")]
                for bb_ in bufs:
                    nc.vector.memset(bb_[64:65, 20, :], 1.0)
                xsrc = x0
                for i in range(n_cross):
                    xdst = bufs[i % 2]
                    if i == 0:
                        cross_layer0_split(x0, xdst)
                    else:
                        cross_layer(i, x0, xsrc, xdst)
                    xsrc = xdst
                xfin = xsrc
                if debug_x:
                    for t in range(KT):
                        dbg = tpool.tile([128, 512], f32, tag="dbg")
                        for ch in range(NCH):
                            nc.vector.tensor_copy(dbg[:], xfin[:, t, ch * 512:(ch + 1) * 512])
                            nc.sync.dma_start(
                                xdbg_d[p, t, :, ch * 512:(ch + 1) * 512], dbg[:])
                if with_mlp:
                    ha = [xpool.tile([128, BC], bf16, tag=f"ha{t}", name=f"ha{t}") for t in range(MT)]
                    hb = [xpool.tile([128, BC], bf16, tag=f"hb{t}", name=f"hb{t}") for t in range(MT)]
                    ha8 = xpool.tile([128, MT, BC], f8, tag="ha8", name="ha8")
                    hb8 = xpool.tile([128, MT, BC], f8, tag="hb8", name="hb8")
                    mlp_w0(x0, ha, ha8)
                    hsrc, hsrc8 = ha, ha8
                    for l in range(N_MLP_HID):
                        hdst = hb if l % 2 == 0 else ha
                        hdst8 = hb8 if l % 2 == 0 else ha8
                        last = l == N_MLP_HID - 1
                        mlp_hidden(l, hsrc8, hdst, None if last else hdst8)
                        hsrc, hsrc8 = hdst, hdst8
                    final_layer(p, xfin, hsrc)

    # Route each SWDGE DMA to the hardware queue matching its DMASW lane.
    # Tile's sem assignment distributes SWDGE DMAs round-robin over 8 DMASW
    # lanes (one vector-clock dim + sem per lane) and assumes completions
    # within a lane are FIFO.  Mapping queue = lane % 4 keeps every lane on
    # a single hardware queue, so in-lane FIFO still holds while the 4
    # queues run descriptor generation in parallel.
    if nc.num_swdge_queues > 1:
        from concourse.tile_scheduler import PROC_NAME_TO_IDX
        sw0 = PROC_NAME_TO_IDX["DMASW0"]
        sw7 = PROC_NAME_TO_IDX["DMASW7"]
        for inst in swdge_insts:
            if hasattr(inst, "ins"):
                inst = inst.ins
            proc = inst.bass_scheduled_proc
            assert proc is not None and sw0 <= proc <= sw7, (inst.name, proc)
            q = (proc - sw0) % nc.num_swdge_queues
            if isinstance(inst, mybir.InstDMAGatherAnt):
                inst.queue_num = q
            elif isinstance(inst, mybir.InstDMACopy):
                inst.queue = f"qPoolDynamic{q if q else ''}"
            else:
                raise AssertionError(f"unexpected SWDGE inst {type(inst)}")

    nc.compile()
    return nc


# needed at module level for the builder
import concourse.bass as bass  # noqa: E402


def _prep_core_inputs(core, user_input, item_input, numeric_feats, categorical_feats,
                      shared):
    r0 = core * B_CORE
    u = user_input[r0:r0 + B_CORE]
    it = item_input[r0:r0 + B_CORE]
    num = numeric_feats[r0:r0 + B_CORE]
    cat = categorical_feats[r0:r0 + B_CORE]

    u_idx = np.ascontiguousarray(u.reshape(16, 128).T).astype(np.int32)
    i_idx = np.ascontiguousarray(it.reshape(16, 128).T).astype(np.int32)

    c_idx = np.zeros((128, N_CAT * 128), np.int16)
    for f in range(N_CAT):
        for p in range(N_PASS):
            seg = cat[p * BC:(p + 1) * BC, f].astype(np.int16)
            blk = seg.reshape(BC // 16, 16).T          # wrap-A: idx i at [i%16, i//16]
            c_idx[:, f * 128 + p * 64: f * 128 + (p + 1) * 64] = np.tile(blk, (8, 1))

    import ml_dtypes
    numT = np.empty((N_NUM + 1, B_CORE), ml_dtypes.bfloat16)
    numT[:N_NUM] = num.T.astype(ml_dtypes.bfloat16)
    numT[N_NUM] = 1.0

    return {
        "u_idx": u_idx, "i_idx": i_idx, "c_idx": c_idx, "numT": numT,
        **shared,
    }


def _prep_shared(num_W, num_b, user_emb, item_emb, cat_tables,
                 Wc, bc, W0, b0, Wh, bh, Wf, bf):
    import ml_dtypes
    ndiag = np.zeros((N_NUM + 1, N_NUM * EMB), np.float32)
    for f in range(N_NUM):
        ndiag[f, f * EMB:(f + 1) * EMB] = num_W[f]
    ndiag[N_NUM] = num_b.reshape(-1)

    cat_pad = np.zeros((N_CAT * CAT_VOCAB, 128), ml_dtypes.bfloat16)
    ct = cat_tables.astype(ml_dtypes.bfloat16)
    for f in range(N_CAT):
        sl = slice(f * CAT_VOCAB, (f + 1) * CAT_VOCAB)
        if f % 2 == 0:   # destination rows 64:128 of the x^T tile
            cat_pad[sl, 64:128] = ct[f]
        else:            # destination rows 0:64
            cat_pad[sl, 0:64] = ct[f]

    bf16 = ml_dtypes.bfloat16
    fp8 = ml_dtypes.float8_e4m3
    return {
        "ndiag": ndiag.astype(bf16),
        "user_emb": np.ascontiguousarray(user_emb, np.float32),
        "item_emb": np.ascontiguousarray(item_emb, np.float32),
        "cat_pad": cat_pad,
        "Wc": np.ascontiguousarray(Wc, np.float32).astype(fp8),
        "bcx": np.ascontiguousarray(bc, np.float32).astype(fp8),
        "W0": np.ascontiguousarray(W0, np.float32).astype(fp8),
        "b0": np.ascontiguousarray(b0, np.float32).reshape(1, MLP).astype(fp8),
        "Wh": np.ascontiguousarray(Wh, np.float32).astype(fp8),
        "bhT": np.ascontiguousarray(np.asarray(bh, np.float32).T),
        "Wfx": np.ascontiguousarray(Wf[:D], np.float32).astype(fp8),
        "Wfh": np.ascontiguousarray(Wf[D:], np.float32).astype(bf16),
        "bf": np.asarray(bf, np.float32).reshape(1, 1).astype(fp8),
    }


def make_in_maps(user_input, item_input, numeric_feats, categorical_feats,
                 user_emb, item_emb, cat_tables, num_W, num_b,
                 Wc, bc, W0, b0, Wh, bh, Wf, bf):
    user_input = np.asarray(user_input).astype(np.int64)
    item_input = np.asarray(item_input).astype(np.int64)
    numeric_feats = np.asarray(numeric_feats, np.float32)
    categorical_feats = np.asarray(categorical_feats).astype(np.int64)
    shared = _prep_shared(np.asarray(num_W, np.float32), np.asarray(num_b, np.float32),
                          np.asarray(user_emb), np.asarray(item_emb),
                          np.asarray(cat_tables, np.float32),
                          np.asarray(Wc), np.asarray(bc), np.asarray(W0),
                          np.asarray(b0), np.asarray(Wh), np.asarray(bh),
                          np.asarray(Wf), np.asarray(bf))
    return [
        _prep_core_inputs(core, user_input, item_input, numeric_feats,
                          categorical_feats, shared)
        for core in range(CORES)
    ]


def get_nc(**flags):
    key = tuple(sorted(flags.items()))
    if key not in _CACHE:
        _CACHE[key] = _build_nc(**flags)
    return _CACHE[key]


def kernel(**inputs) -> np.ndarray:
    from concourse.bass_utils import run_bass_kernel_spmd
    nc = get_nc()
    in_maps = make_in_maps(**inputs)
    res = run_bass_kernel_spmd(nc, in_maps, list(range(CORES)))
    out = np.concatenate([res.results[i]["out"][0] for i in range(CORES)])
    return out.reshape(B, 1).astype(np.float32)



# revision 45
# speedup vs baseline: 1.4293x; 1.0044x over previous
"""DCNv2 (nn_DCNv2_63462436765991) Trainium2 Bass kernel.

Strategy: pure data-parallel over the batch across 8 NeuronCores
(2048 rows/core).  Per core the model runs in 2 passes of 1024 rows.
Activations live in SBUF feature-major as ONE contiguous fp8e4 buffer
per x family ([128 part, 21 k-tiles, 1024 batch]); cross-network and
W0 matmuls run in fp8 with DoubleRow perf mode (2 k-tiles / 256
contraction rows per instruction, 2x PE throughput), fp32 PSUM
accumulation.  Hidden MLP layers and the final matvec stay bf16
(h activations bf16; xfin is cast fp8->bf16 per chunk for the final
layer).  Numerics validated against the fp32 reference: max rel err
~2.9e-3 (gate 2e-2).  Biases fold via an appended ones-row (cross/W0/
final) or the ACT bias port (hidden layers).

Embedding gathers:
  - categorical: dma_gather(transpose=True) over host-padded bf16
    tables ([10000, 128] rows, 256B each; real data in the column half
    matching the feature's destination partition range) into bf16
    staging, then a DVE cast of the 64-row half into the fp8 x buffer.
  - user/item (vocab 100k > int16): indirect_dma_start, one index per
    partition (batch-major staging), then PE transpose + fp8 cast.

x0 row layout (feature-major):  rows 0:64 user, 64:128 item,
128:960 numeric (13 x 64), 960:2624 categorical (26 x 64).
"""

import numpy as np

B = 16384
CORES = 8
B_CORE = B // CORES            # 2048
N_PASS = 2
BC = B_CORE // N_PASS          # 1024 batch per pass
NCH = BC // 512                # matmul N-chunks per pass
EMB = 64
N_NUM = 13
N_CAT = 26
CAT_VOCAB = 10000
D = 2624
KT = 21                        # k-tiles over D (20 x 128 + 64)
MLP = 1024
MT = MLP // 128                # 8
L_CROSS = 4
N_MLP_HID = 3

_CACHE = {}


def _build_nc(n_cross=L_CROSS, with_mlp=True, debug_x0=False, debug_x=False,
              parts=("cat", "num", "uit"), repeats=1):
    import concourse.bass as bass
    import concourse.mybir as mybir
    import concourse.tile as tile
    from concourse import bacc
    from concourse.masks import make_identity

    f32 = mybir.dt.float32
    bf16 = mybir.dt.bfloat16
    f8 = mybir.dt.float8e4
    i32 = mybir.dt.int32
    i16 = mybir.dt.int16
    DR = mybir.MatmulPerfMode.DoubleRow
    MULT = mybir.AluOpType.mult
    ADD = mybir.AluOpType.add
    RELU = mybir.ActivationFunctionType.Relu
    COPY = mybir.ActivationFunctionType.Copy
    SIGM = mybir.ActivationFunctionType.Sigmoid

    # NOTE: num_swdge_queues>1 was tried (parallel gather descgen, −185us
    # device time) but produces nondeterministic wrong gather data on HW
    # even with DMASW-lane-consistent queue assignment — reverted.
    nc = bacc.Bacc("TRN2", target_bir_lowering=False, debug=False,
                   num_swdge_queues=1)
    # SWDGE DMA instructions (gathers / indirects) collected at emission;
    # their hardware queue is assigned post-scheduling from the DMASW lane.
    swdge_insts = []

    # ---- DRAM I/O ----
    u_idx_d = nc.dram_tensor("u_idx", [128, 16], i32, kind="ExternalInput")
    i_idx_d = nc.dram_tensor("i_idx", [128, 16], i32, kind="ExternalInput")
    c_idx_d = nc.dram_tensor("c_idx", [128, N_CAT * 128], i16, kind="ExternalInput")
    numT_d = nc.dram_tensor("numT", [N_NUM + 1, B_CORE], bf16, kind="ExternalInput")
    ndiag_d = nc.dram_tensor("ndiag", [N_NUM + 1, N_NUM * EMB], bf16, kind="ExternalInput")
    uemb_d = nc.dram_tensor("user_emb", [100000, EMB], f32, kind="ExternalInput")
    iemb_d = nc.dram_tensor("item_emb", [100000, EMB], f32, kind="ExternalInput")
    cpad_d = nc.dram_tensor("cat_pad", [N_CAT * CAT_VOCAB, 128], bf16, kind="ExternalInput")
    Wc_d = nc.dram_tensor("Wc", [L_CROSS, D, D], f8, kind="ExternalInput")
    bc_d = nc.dram_tensor("bcx", [L_CROSS, D], f8, kind="ExternalInput")
    W0_d = nc.dram_tensor("W0", [D, MLP], f8, kind="ExternalInput")
    b0_d = nc.dram_tensor("b0", [1, MLP], f8, kind="ExternalInput")
    Wh_d = nc.dram_tensor("Wh", [N_MLP_HID, MLP, MLP], f8, kind="ExternalInput")
    bhT_d = nc.dram_tensor("bhT", [MLP, N_MLP_HID], f32, kind="ExternalInput")
    Wfx_d = nc.dram_tensor("Wfx", [D, 1], f8, kind="ExternalInput")
    Wfh_d = nc.dram_tensor("Wfh", [MLP, 1], bf16, kind="ExternalInput")
    bf_d = nc.dram_tensor("bf", [1, 1], f8, kind="ExternalInput")
    out_d = nc.dram_tensor("out", [1, B_CORE], f32, kind="ExternalOutput")
    if debug_x0:
        x0dbg_d = nc.dram_tensor("x0dbg", [N_PASS, KT, 128, BC], f32, kind="ExternalOutput")
    if debug_x:
        xdbg_d = nc.dram_tensor("xdbg", [N_PASS, KT, 128, BC], f32, kind="ExternalOutput")

    with tile.TileContext(nc) as tc:
        from contextlib import ExitStack
        with ExitStack() as ctx:
            const = ctx.enter_context(tc.tile_pool(name="const", bufs=1))
            xpool = ctx.enter_context(tc.tile_pool(name="xpool", bufs=1))
            wpool = ctx.enter_context(tc.tile_pool(name="wpool", bufs=2))
            stpool = ctx.enter_context(tc.tile_pool(name="stpool", bufs=2))
            tpool = ctx.enter_context(tc.tile_pool(name="tpool", bufs=4))
            bpool = ctx.enter_context(tc.tile_pool(name="bpool", bufs=2))
            zpool = ctx.enter_context(tc.tile_pool(name="zpool", bufs=2))
            mmps = ctx.enter_context(tc.tile_pool(name="mmps", bufs=4, space="PSUM"))
            trps = ctx.enter_context(tc.tile_pool(name="trps", bufs=2, space="PSUM"))

            # ---- per-core constants ----
            uidx = const.tile([128, 16], i32)
            iidx = const.tile([128, 16], i32)
            cidx = const.tile([128, N_CAT * 128], i16)
            numT = const.tile([N_NUM + 1, B_CORE], bf16)
            ndiag = const.tile([N_NUM + 1, N_NUM * EMB], bf16)
            ident = const.tile([128, 128], f32)
            nc.sync.dma_start(uidx[:], u_idx_d[:])
            nc.sync.dma_start(iidx[:], i_idx_d[:])
            nc.sync.dma_start(cidx[:], c_idx_d[:])
            nc.sync.dma_start(numT[:], numT_d[:])
            nc.sync.dma_start(ndiag[:], ndiag_d[:])
            make_identity(nc, ident)

            def alloc_x(prefix):
                # one contiguous fp8 buffer [128, KT, BC]: k-tile t at [:, t, :]
                return xpool.tile([128, KT, BC], f8, tag=prefix, name=prefix)

            def assemble_x0(p, x0):
                # --- user/item: indirect gather issued first so their tile-0
                # data lands early; hardware queue patched post-scheduling ---
                if "uit" in parts:
                    stu = stpool.tile([128, 8, 2, EMB], f32, tag="uit")
                    for c in range(8):
                        pc = p * 8 + c
                        swdge_insts.append(nc.gpsimd.indirect_dma_start(
                            out=stu[:, c, 0, :], out_offset=None, in_=uemb_d[:],
                            in_offset=bass.IndirectOffsetOnAxis(ap=uidx[:, pc:pc + 1], axis=0)))
                        swdge_insts.append(nc.gpsimd.indirect_dma_start(
                            out=stu[:, c, 1, :], out_offset=None, in_=iemb_d[:],
                            in_offset=bass.IndirectOffsetOnAxis(ap=iidx[:, pc:pc + 1], axis=0)))

                # --- categorical gathers (dma_gather transpose): stage bf16,
                # cast the feature's 64-row half into the fp8 x buffer ---
                for f in range(N_CAT if "cat" in parts else 0):
                    trow = 960 + 64 * f
                    t, off = divmod(trow, 128)
                    idx_ap = cidx[:, f * 128 + p * 64: f * 128 + p * 64 + 64]
                    stg = stpool.tile([128, 1, BC], bf16, tag="cstg")
                    swdge_insts.append(nc.gpsimd.dma_gather(
                        out_ap=stg[:], in_ap=cpad_d[f * CAT_VOCAB:(f + 1) * CAT_VOCAB, :],
                        idxs_ap=idx_ap, num_idxs=BC, num_idxs_reg=BC,
                        elem_size=128, transpose=True, single_packet=False))
                    # cast on the Scalar engine: DVE saturates during cross
                    nc.scalar.activation(x0[off:off + 64, t, :], stg[off:off + 64, 0, :], COPY)
                # ones row for the bias fold
                nc.vector.memset(x0[64:65, 20, :], 1.0)

                # --- numeric features: diag-expanded matmul ---
                for m in range(7 if "num" in parts else 0):
                    mw = 128 if m < 6 else 64
                    for ch in range(NCH):
                        ps = mmps.tile([128, 512], mybir.dt.float32, space="PSUM", tag="psacc")
                        nc.tensor.matmul(
                            ps[:mw, :], ndiag[:, m * 128: m * 128 + mw],
                            numT[:, p * BC + ch * 512: p * BC + (ch + 1) * 512],
                            start=True, stop=True)
                        if m < 6:
                            dst = x0[:, 1 + m, ch * 512:(ch + 1) * 512]
                        else:
                            dst = x0[0:64, 7, ch * 512:(ch + 1) * 512]
                        nc.scalar.activation(dst, ps[:mw, :], COPY)

                # --- user/item: PE transpose of the staged rows ---
                if "uit" not in parts:
                    return
                for c in range(8):
                    pst = trps.tile([128, 128], f32, space="PSUM")
                    nc.tensor.transpose(pst[:], stu[:, c, :, :], ident[:])
                    nc.scalar.activation(x0[:, 0, c * 128:(c + 1) * 128], pst[:], COPY)

            def dense_layer(w_src, b_src, xsrc, j, jw, evict):
                """One output j-tile over the 21 fp8 k-tiles of xsrc.

                k-tiles 0..19 run as 10 DoubleRow pair-matmuls (2 k-tiles,
                256 contraction rows per instruction); the 64-row tail plus
                the bias ones-row run as one normal fp8 matmul.
                """
                j0 = j * 128
                wmain = wpool.tile([128, 20, 128], f8, tag="wmain")
                nc.sync.dma_start(
                    wmain[:, :, :jw],
                    w_src[0:2560, j0:j0 + jw]
                    .rearrange("(ko q) n -> q ko n", q=128))
                pss = [mmps.tile([128, 512], mybir.dt.float32, space="PSUM",
                                 name="psacc", tag="psacc") for _ in range(NCH)]
                wlast = wpool.tile([128, 128], f8, tag="wlast")
                nc.sync.dma_start(wlast[0:64, :jw], w_src[2560:2624, j0:j0 + jw])
                nc.sync.dma_start(wlast[64:65, :jw], b_src[0:1, j0:j0 + jw])
                for t in range(10):
                    for ch in range(NCH):
                        nc.tensor.matmul(
                            pss[ch][:jw, :], wmain[:, 2 * t:2 * t + 2, :jw],
                            xsrc[:, 2 * t:2 * t + 2, ch * 512:(ch + 1) * 512],
                            start=(t == 0), stop=False, perf_mode=DR)
                for ch in range(NCH):
                    nc.tensor.matmul(
                        pss[ch][:jw, :], wlast[0:65, :jw],
                        xsrc[0:65, 20, ch * 512:(ch + 1) * 512],
                        start=False, stop=True)
                for ch in range(NCH):
                    evict(pss[ch], ch)

            def cross_layer(i, x0, xsrc, xdst):
                for j in range(KT):
                    jw = 128 if j < 20 else 64

                    def evict(ps, ch, j=j, jw=jw):
                        sl = slice(ch * 512, (ch + 1) * 512)
                        tmp = tpool.tile([128, 512], bf16, tag="evt")
                        nc.vector.tensor_tensor(tmp[:jw, :], ps[:jw, :], x0[:jw, j, sl], MULT)
                        nc.vector.tensor_tensor(xdst[:jw, j, sl], tmp[:jw, :], xsrc[:jw, j, sl], ADD)

                    dense_layer(Wc_d[i], bc_d[i:i + 1], xsrc, j, jw, evict)

            def mlp_w0(x0, ha, ha8):
                for j in range(MT):
                    def evict(ps, ch, j=j):
                        sl = slice(ch * 512, (ch + 1) * 512)
                        nc.scalar.activation(ha[j][:, sl], ps[:, :], RELU)
                        nc.vector.tensor_copy(ha8[:, j, sl], ha[j][:, sl])
                    dense_layer(W0_d[:], b0_d[:], x0, j, 128, evict)

            def mlp_hidden(l, src8, dst, dst8):
                for j in range(MT):
                    j0 = j * 128
                    whm = wpool.tile([128, MT, 128], f8, tag="whid")
                    nc.sync.dma_start(
                        whm[:, :, :], Wh_d[l, :, j0:j0 + 128]
                        .rearrange("(ko q) n -> q ko n", q=128))
                    bias = bpool.tile([128, 1], f32, tag="bias")
                    nc.sync.dma_start(bias[:], bhT_d[j0:j0 + 128, l:l + 1])
                    pss = [mmps.tile([128, 512], mybir.dt.float32, space="PSUM",
                                     name="psacc", tag="psacc") for _ in range(NCH)]
                    for t in range(MT // 2):
                        for ch in range(NCH):
                            nc.tensor.matmul(
                                pss[ch][:, :], whm[:, 2 * t:2 * t + 2, :],
                                src8[:, 2 * t:2 * t + 2, ch * 512:(ch + 1) * 512],
                                start=(t == 0), stop=(t == MT // 2 - 1),
                                perf_mode=DR)
                    for ch in range(NCH):
                        sl = slice(ch * 512, (ch + 1) * 512)
                        nc.scalar.activation(dst[j][:, sl], pss[ch][:, :],
                                             RELU, bias=bias[:])
                        if dst8 is not None:
                            nc.vector.tensor_copy(dst8[:, j, sl], dst[j][:, sl])

            def final_layer(p, xfin, hfin):
                wfm = wpool.tile([128, 20, 1], f8, tag="wfm")
                nc.sync.dma_start(
                    wfm[:], Wfx_d[0:2560, 0:1].rearrange("(ko q) n -> q ko n", q=128))
                wfl = wpool.tile([128, 1], f8, tag="wfl")
                nc.sync.dma_start(wfl[0:64, :], Wfx_d[2560:2624, 0:1])
                nc.sync.dma_start(wfl[64:65, :], bf_d[:])
                wfh = wpool.tile([128, MT, 1], bf16, tag="wfh")
                nc.sync.dma_start(
                    wfh[:], Wfh_d[:, 0:1].rearrange("(ko q) n -> q ko n", q=128))
                for ch in range(NCH):
                    sl = slice(ch * 512, (ch + 1) * 512)
                    zps = mmps.tile([128, 512], mybir.dt.float32, space="PSUM", tag="psacc")
                    for t in range(20):
                        nc.tensor.matmul(zps[0:1, :], wfm[:, t, :], xfin[:, t, sl],
                                         start=(t == 0), stop=False)
                    nc.tensor.matmul(zps[0:1, :], wfl[0:65, :], xfin[0:65, 20, sl],
                                     start=False, stop=False)
                    for t in range(MT):
                        nc.tensor.matmul(zps[0:1, :], wfh[:, t, :],
                                         hfin[t][:, sl],
                                         start=False, stop=(t == MT - 1))
                    zsb = zpool.tile([1, 512], f32, tag="zsb")
                    nc.scalar.activation(zsb[:], zps[0:1, :], SIGM)
                    nc.sync.dma_start(
                        out_d[0:1, p * BC + ch * 512: p * BC + (ch + 1) * 512], zsb[:])

            for pi, p in enumerate([pp for _ in range(repeats) for pp in range(N_PASS)]):
                # Pass p's x0 gets its own family (0 or 3) so pass p+1's
                # gathers start immediately instead of WAR-waiting on the
                # cross ping-pong buffers; ping-pong uses families 1/2.
                x0fam = 0 if pi % 2 == 0 else 3
                afam = 1
                x0 = alloc_x(f"xs{x0fam}_")
                assemble_x0(p, x0)
                if debug_x0:
                    for t in range(KT):
                        dbg = tpool.tile([128, 512], f32, tag="dbg")
                        for ch in range(NCH):
                            nc.vector.tensor_copy(dbg[:], x0[:, t, ch * 512:(ch + 1) * 512])
                            nc.sync.dma_start(
                                x0dbg_d[p, t, :, ch * 512:(ch + 1) * 512], dbg[:])
                bufs = [alloc_x(f"xs{afam}_"), alloc_x("xs2_")]
                for bb_ in bufs:
                    nc.vector.memset(bb_[64:65, 20, :], 1.0)
                xsrc = x0
                for i in range(n_cross):
                    xdst = bufs[i % 2]
                    cross_layer(i, x0, xsrc, xdst)
                    xsrc = xdst
                xfin = xsrc
                if debug_x:
                    for t in range(KT):
                        dbg = tpool.tile([128, 512], f32, tag="dbg")
                        for ch in range(NCH):
                            nc.vector.tensor_copy(dbg[:], xfin[:, t, ch * 512:(ch + 1) * 512])
                            nc.sync.dma_start(
                                xdbg_d[p, t, :, ch * 512:(ch + 1) * 512], dbg[:])
                if with_mlp:
                    ha = [xpool.tile([128, BC], bf16, tag=f"ha{t}", name=f"ha{t}") for t in range(MT)]
                    hb = [xpool.tile([128, BC], bf16, tag=f"hb{t}", name=f"hb{t}") for t in range(MT)]
                    ha8 = xpool.tile([128, MT, BC], f8, tag="ha8", name="ha8")
                    hb8 = xpool.tile([128, MT, BC], f8, tag="hb8", name="hb8")
                    mlp_w0(x0, ha, ha8)
                    hsrc, hsrc8 = ha, ha8
                    for l in range(N_MLP_HID):
                        hdst = hb if l % 2 == 0 else ha
                        hdst8 = hb8 if l % 2 == 0 else ha8
                        last = l == N_MLP_HID - 1
                        mlp_hidden(l, hsrc8, hdst, None if last else hdst8)
                        hsrc, hsrc8 = hdst, hdst8
                    final_layer(p, xfin, hsrc)

    # Route each SWDGE DMA to the hardware queue matching its DMASW lane.
    # Tile's sem assignment distributes SWDGE DMAs round-robin over 8 DMASW
    # lanes (one vector-clock dim + sem per lane) and assumes completions
    # within a lane are FIFO.  Mapping queue = lane % 4 keeps every lane on
    # a single hardware queue, so in-lane FIFO still holds while the 4
    # queues run descriptor generation in parallel.
    if nc.num_swdge_queues > 1:
        from concourse.tile_scheduler import PROC_NAME_TO_IDX
        sw0 = PROC_NAME_TO_IDX["DMASW0"]
        sw7 = PROC_NAME_TO_IDX["DMASW7"]
        for inst in swdge_insts:
            if hasattr(inst, "ins"):
                inst = inst.ins
            proc = inst.bass_scheduled_proc
            assert proc is not None and sw0 <= proc <= sw7, (inst.name, proc)
            q = (proc - sw0) % nc.num_swdge_queues
            if isinstance(inst, mybir.InstDMAGatherAnt):
                inst.queue_num = q
            elif isinstance(inst, mybir.InstDMACopy):
                inst.queue = f"qPoolDynamic{q if q else ''}"
            else:
                raise AssertionError(f"unexpected SWDGE inst {type(inst)}")

    nc.compile()
    return nc


# needed at module level for the builder
import concourse.bass as bass  # noqa: E402


def _prep_core_inputs(core, user_input, item_input, numeric_feats, categorical_feats,
                      shared):
    r0 = core * B_CORE
    u = user_input[r0:r0 + B_CORE]
    it = item_input[r0:r0 + B_CORE]
    num = numeric_feats[r0:r0 + B_CORE]
    cat = categorical_feats[r0:r0 + B_CORE]

    u_idx = np.ascontiguousarray(u.reshape(16, 128).T).astype(np.int32)
    i_idx = np.ascontiguousarray(it.reshape(16, 128).T).astype(np.int32)

    c_idx = np.zeros((128, N_CAT * 128), np.int16)
    for f in range(N_CAT):
        for p in range(N_PASS):
            seg = cat[p * BC:(p + 1) * BC, f].astype(np.int16)
            blk = seg.reshape(BC // 16, 16).T          # wrap-A: idx i at [i%16, i//16]
            c_idx[:, f * 128 + p * 64: f * 128 + (p + 1) * 64] = np.tile(blk, (8, 1))

    import ml_dtypes
    numT = np.empty((N_NUM + 1, B_CORE), ml_dtypes.bfloat16)
    numT[:N_NUM] = num.T.astype(ml_dtypes.bfloat16)
    numT[N_NUM] = 1.0

    return {
        "u_idx": u_idx, "i_idx": i_idx, "c_idx": c_idx, "numT": numT,
        **shared,
    }


def _prep_shared(num_W, num_b, user_emb, item_emb, cat_tables,
                 Wc, bc, W0, b0, Wh, bh, Wf, bf):
    import ml_dtypes
    ndiag = np.zeros((N_NUM + 1, N_NUM * EMB), np.float32)
    for f in range(N_NUM):
        ndiag[f, f * EMB:(f + 1) * EMB] = num_W[f]
    ndiag[N_NUM] = num_b.reshape(-1)

    cat_pad = np.zeros((N_CAT * CAT_VOCAB, 128), ml_dtypes.bfloat16)
    ct = cat_tables.astype(ml_dtypes.bfloat16)
    for f in range(N_CAT):
        sl = slice(f * CAT_VOCAB, (f + 1) * CAT_VOCAB)
        if f % 2 == 0:   # destination rows 64:128 of the x^T tile
            cat_pad[sl, 64:128] = ct[f]
        else:            # destination rows 0:64
            cat_pad[sl, 0:64] = ct[f]

    bf16 = ml_dtypes.bfloat16
    fp8 = ml_dtypes.float8_e4m3
    return {
        "ndiag": ndiag.astype(bf16),
        "user_emb": np.ascontiguousarray(user_emb, np.float32),
        "item_emb": np.ascontiguousarray(item_emb, np.float32),
        "cat_pad": cat_pad,
        "Wc": np.ascontiguousarray(Wc, np.float32).astype(fp8),
        "bcx": np.ascontiguousarray(bc, np.float32).astype(fp8),
        "W0": np.ascontiguousarray(W0, np.float32).astype(fp8),
        "b0": np.ascontiguousarray(b0, np.float32).reshape(1, MLP).astype(fp8),
        "Wh": np.ascontiguousarray(Wh, np.float32).astype(fp8),
        "bhT": np.ascontiguousarray(np.asarray(bh, np.float32).T),
        "Wfx": np.ascontiguousarray(Wf[:D], np.float32).astype(fp8),
        "Wfh": np.ascontiguousarray(Wf[D:], np.float32).astype(bf16),
        "bf": np.asarray(bf, np.float32).reshape(1, 1).astype(fp8),
    }


def make_in_maps(user_input, item_input, numeric_feats, categorical_feats,
                 user_emb, item_emb, cat_tables, num_W, num_b,
                 Wc, bc, W0, b0, Wh, bh, Wf, bf):
    user_input = np.asarray(user_input).astype(np.int64)
    item_input = np.asarray(item_input).astype(np.int64)
    numeric_feats = np.asarray(numeric_feats, np.float32)
    categorical_feats = np.asarray(categorical_feats).astype(np.int64)
    shared = _prep_shared(np.asarray(num_W, np.float32), np.asarray(num_b, np.float32),
                          np.asarray(user_emb), np.asarray(item_emb),
                          np.asarray(cat_tables, np.float32),
                          np.asarray(Wc), np.asarray(bc), np.asarray(W0),
                          np.asarray(b0), np.asarray(Wh), np.asarray(bh),
                          np.asarray(Wf), np.asarray(bf))
    return [
        _prep_core_inputs(core, user_input, item_input, numeric_feats,
                          categorical_feats, shared)
        for core in range(CORES)
    ]


def get_nc(**flags):
    key = tuple(sorted(flags.items()))
    if key not in _CACHE:
        _CACHE[key] = _build_nc(**flags)
    return _CACHE[key]


def kernel(**inputs) -> np.ndarray:
    from concourse.bass_utils import run_bass_kernel_spmd
    nc = get_nc()
    in_maps = make_in_maps(**inputs)
    res = run_bass_kernel_spmd(nc, in_maps, list(range(CORES)))
    out = np.concatenate([res.results[i]["out"][0] for i in range(CORES)])
    return out.reshape(B, 1).astype(np.float32)



# revision 46
# speedup vs baseline: 1.4429x; 1.0095x over previous
"""DCNv2 (nn_DCNv2_63462436765991) Trainium2 Bass kernel.

Strategy: pure data-parallel over the batch across 8 NeuronCores
(2048 rows/core).  Per core the model runs in 2 passes of 1024 rows.
Activations live in SBUF feature-major as ONE contiguous fp8e4 buffer
per x family ([128 part, 21 k-tiles, 1024 batch]); cross-network and
W0 matmuls run in fp8 with DoubleRow perf mode (2 k-tiles / 256
contraction rows per instruction, 2x PE throughput), fp32 PSUM
accumulation.  Hidden MLP layers and the final matvec stay bf16
(h activations bf16; xfin is cast fp8->bf16 per chunk for the final
layer).  Numerics validated against the fp32 reference: max rel err
~2.9e-3 (gate 2e-2).  Biases fold via an appended ones-row (cross/W0/
final) or the ACT bias port (hidden layers).

Embedding gathers:
  - categorical: dma_gather(transpose=True) over host-padded bf16
    tables ([10000, 128] rows, 256B each; real data in the column half
    matching the feature's destination partition range) into bf16
    staging, then a DVE cast of the 64-row half into the fp8 x buffer.
  - user/item (vocab 100k > int16): indirect_dma_start, one index per
    partition (batch-major staging), then PE transpose + fp8 cast.

x0 row layout (feature-major):  rows 0:64 user, 64:128 item,
128:960 numeric (13 x 64), 960:2624 categorical (26 x 64).
"""

import numpy as np

B = 16384
CORES = 8
B_CORE = B // CORES            # 2048
N_PASS = 2
BC = B_CORE // N_PASS          # 1024 batch per pass
NCH = BC // 512                # matmul N-chunks per pass
EMB = 64
N_NUM = 13
N_CAT = 26
CAT_VOCAB = 10000
D = 2624
KT = 21                        # k-tiles over D (20 x 128 + 64)
MLP = 1024
MT = MLP // 128                # 8
L_CROSS = 4
N_MLP_HID = 3

_CACHE = {}


def _build_nc(n_cross=L_CROSS, with_mlp=True, debug_x0=False, debug_x=False,
              parts=("cat", "num", "uit"), repeats=1):
    import concourse.bass as bass
    import concourse.mybir as mybir
    import concourse.tile as tile
    from concourse import bacc
    from concourse.masks import make_identity

    f32 = mybir.dt.float32
    bf16 = mybir.dt.bfloat16
    f8 = mybir.dt.float8e4
    i32 = mybir.dt.int32
    i16 = mybir.dt.int16
    DR = mybir.MatmulPerfMode.DoubleRow
    MULT = mybir.AluOpType.mult
    ADD = mybir.AluOpType.add
    RELU = mybir.ActivationFunctionType.Relu
    COPY = mybir.ActivationFunctionType.Copy
    SIGM = mybir.ActivationFunctionType.Sigmoid

    # NOTE: num_swdge_queues>1 was tried (parallel gather descgen, −185us
    # device time) but produces nondeterministic wrong gather data on HW
    # even with DMASW-lane-consistent queue assignment — reverted.
    nc = bacc.Bacc("TRN2", target_bir_lowering=False, debug=False,
                   num_swdge_queues=1)
    # SWDGE DMA instructions (gathers / indirects) collected at emission;
    # their hardware queue is assigned post-scheduling from the DMASW lane.
    swdge_insts = []

    # ---- DRAM I/O ----
    u_idx_d = nc.dram_tensor("u_idx", [128, 16], i32, kind="ExternalInput")
    i_idx_d = nc.dram_tensor("i_idx", [128, 16], i32, kind="ExternalInput")
    c_idx_d = nc.dram_tensor("c_idx", [128, N_CAT * 128], i16, kind="ExternalInput")
    numT_d = nc.dram_tensor("numT", [N_NUM + 1, B_CORE], bf16, kind="ExternalInput")
    ndiag_d = nc.dram_tensor("ndiag", [N_NUM + 1, N_NUM * EMB], bf16, kind="ExternalInput")
    uemb_d = nc.dram_tensor("user_emb", [100000, EMB], f32, kind="ExternalInput")
    iemb_d = nc.dram_tensor("item_emb", [100000, EMB], f32, kind="ExternalInput")
    cpad_d = nc.dram_tensor("cat_pad", [N_CAT * CAT_VOCAB, 128], bf16, kind="ExternalInput")
    Wc_d = nc.dram_tensor("Wc", [L_CROSS, D, D], f8, kind="ExternalInput")
    bc_d = nc.dram_tensor("bcx", [L_CROSS, D], f8, kind="ExternalInput")
    W0_d = nc.dram_tensor("W0", [D, MLP], f8, kind="ExternalInput")
    b0_d = nc.dram_tensor("b0", [1, MLP], f8, kind="ExternalInput")
    Wh_d = nc.dram_tensor("Wh", [N_MLP_HID, MLP, MLP], f8, kind="ExternalInput")
    bhT_d = nc.dram_tensor("bhT", [MLP, N_MLP_HID], f32, kind="ExternalInput")
    Wfx_d = nc.dram_tensor("Wfx", [D, 1], f8, kind="ExternalInput")
    Wfh_d = nc.dram_tensor("Wfh", [MLP, 1], bf16, kind="ExternalInput")
    bf_d = nc.dram_tensor("bf", [1, 1], f8, kind="ExternalInput")
    out_d = nc.dram_tensor("out", [1, B_CORE], f32, kind="ExternalOutput")
    if debug_x0:
        x0dbg_d = nc.dram_tensor("x0dbg", [N_PASS, KT, 128, BC], f32, kind="ExternalOutput")
    if debug_x:
        xdbg_d = nc.dram_tensor("xdbg", [N_PASS, KT, 128, BC], f32, kind="ExternalOutput")

    with tile.TileContext(nc) as tc:
        from contextlib import ExitStack
        with ExitStack() as ctx:
            const = ctx.enter_context(tc.tile_pool(name="const", bufs=1))
            xpool = ctx.enter_context(tc.tile_pool(name="xpool", bufs=1))
            wpool = ctx.enter_context(tc.tile_pool(name="wpool", bufs=2))
            stpool = ctx.enter_context(tc.tile_pool(name="stpool", bufs=2))
            tpool = ctx.enter_context(tc.tile_pool(name="tpool", bufs=4))
            bpool = ctx.enter_context(tc.tile_pool(name="bpool", bufs=2))
            zpool = ctx.enter_context(tc.tile_pool(name="zpool", bufs=2))
            mmps = ctx.enter_context(tc.tile_pool(name="mmps", bufs=4, space="PSUM"))
            trps = ctx.enter_context(tc.tile_pool(name="trps", bufs=2, space="PSUM"))

            # ---- per-core constants ----
            uidx = const.tile([128, 16], i32)
            iidx = const.tile([128, 16], i32)
            cidx = const.tile([128, N_CAT * 128], i16)
            numT = const.tile([N_NUM + 1, B_CORE], bf16)
            ndiag = const.tile([N_NUM + 1, N_NUM * EMB], bf16)
            ident = const.tile([128, 128], f32)
            nc.sync.dma_start(uidx[:], u_idx_d[:])
            nc.sync.dma_start(iidx[:], i_idx_d[:])
            nc.sync.dma_start(cidx[:], c_idx_d[:])
            nc.sync.dma_start(numT[:], numT_d[:])
            nc.sync.dma_start(ndiag[:], ndiag_d[:])
            make_identity(nc, ident)

            def alloc_x(prefix):
                # one contiguous fp8 buffer [128, KT, BC]: k-tile t at [:, t, :]
                return xpool.tile([128, KT, BC], f8, tag=prefix, name=prefix)

            def assemble_x0(p, x0):
                # --- user/item: indirect gather issued first so their tile-0
                # data lands early; hardware queue patched post-scheduling ---
                if "uit" in parts:
                    stu = stpool.tile([128, 8, 2, EMB], f32, tag="uit")
                    for c in range(8):
                        pc = p * 8 + c
                        swdge_insts.append(nc.gpsimd.indirect_dma_start(
                            out=stu[:, c, 0, :], out_offset=None, in_=uemb_d[:],
                            in_offset=bass.IndirectOffsetOnAxis(ap=uidx[:, pc:pc + 1], axis=0)))
                        swdge_insts.append(nc.gpsimd.indirect_dma_start(
                            out=stu[:, c, 1, :], out_offset=None, in_=iemb_d[:],
                            in_offset=bass.IndirectOffsetOnAxis(ap=iidx[:, pc:pc + 1], axis=0)))

                # --- categorical gathers (dma_gather transpose): stage bf16,
                # cast the feature's 64-row half into the fp8 x buffer ---
                for f in range(N_CAT if "cat" in parts else 0):
                    trow = 960 + 64 * f
                    t, off = divmod(trow, 128)
                    idx_ap = cidx[:, f * 128 + p * 64: f * 128 + p * 64 + 64]
                    stg = stpool.tile([128, 1, BC], bf16, tag="cstg")
                    swdge_insts.append(nc.gpsimd.dma_gather(
                        out_ap=stg[:], in_ap=cpad_d[f * CAT_VOCAB:(f + 1) * CAT_VOCAB, :],
                        idxs_ap=idx_ap, num_idxs=BC, num_idxs_reg=BC,
                        elem_size=128, transpose=True, single_packet=False))
                    # cast on the Scalar engine: DVE saturates during cross
                    nc.scalar.activation(x0[off:off + 64, t, :], stg[off:off + 64, 0, :], COPY)
                # ones row for the bias fold
                nc.vector.memset(x0[64:65, 20, :], 1.0)

                # --- numeric features: diag-expanded matmul ---
                for m in range(7 if "num" in parts else 0):
                    mw = 128 if m < 6 else 64
                    for ch in range(NCH):
                        ps = mmps.tile([128, 512], mybir.dt.float32, space="PSUM", tag="psacc")
                        nc.tensor.matmul(
                            ps[:mw, :], ndiag[:, m * 128: m * 128 + mw],
                            numT[:, p * BC + ch * 512: p * BC + (ch + 1) * 512],
                            start=True, stop=True)
                        if m < 6:
                            dst = x0[:, 1 + m, ch * 512:(ch + 1) * 512]
                        else:
                            dst = x0[0:64, 7, ch * 512:(ch + 1) * 512]
                        nc.scalar.activation(dst, ps[:mw, :], COPY)

                # --- user/item: PE transpose of the staged rows ---
                if "uit" not in parts:
                    return
                for c in range(8):
                    pst = trps.tile([128, 128], f32, space="PSUM")
                    nc.tensor.transpose(pst[:], stu[:, c, :, :], ident[:])
                    nc.scalar.activation(x0[:, 0, c * 128:(c + 1) * 128], pst[:], COPY)

            def dense_layer(w_src, b_src, xsrc, j, jw, evict):
                """One output j-tile over the 21 fp8 k-tiles of xsrc.

                k-tiles 0..19 run as 10 DoubleRow pair-matmuls (2 k-tiles,
                256 contraction rows per instruction); the 64-row tail plus
                the bias ones-row run as one normal fp8 matmul.
                """
                j0 = j * 128
                wmain = wpool.tile([128, 20, 128], f8, tag="wmain")
                nc.sync.dma_start(
                    wmain[:, :, :jw],
                    w_src[0:2560, j0:j0 + jw]
                    .rearrange("(ko q) n -> q ko n", q=128))
                pss = [mmps.tile([128, 512], mybir.dt.float32, space="PSUM",
                                 name="psacc", tag="psacc") for _ in range(NCH)]
                wlast = wpool.tile([128, 128], f8, tag="wlast")
                nc.sync.dma_start(wlast[0:64, :jw], w_src[2560:2624, j0:j0 + jw])
                nc.sync.dma_start(wlast[64:65, :jw], b_src[0:1, j0:j0 + jw])
                for t in range(10):
                    for ch in range(NCH):
                        nc.tensor.matmul(
                            pss[ch][:jw, :], wmain[:, 2 * t:2 * t + 2, :jw],
                            xsrc[:, 2 * t:2 * t + 2, ch * 512:(ch + 1) * 512],
                            start=(t == 0), stop=False, perf_mode=DR)
                for ch in range(NCH):
                    nc.tensor.matmul(
                        pss[ch][:jw, :], wlast[0:65, :jw],
                        xsrc[0:65, 20, ch * 512:(ch + 1) * 512],
                        start=False, stop=True)
                for ch in range(NCH):
                    evict(pss[ch], ch)

            def cross_layer(i, x0, xsrc, xdst):
                for j in range(KT):
                    jw = 128 if j < 20 else 64

                    def evict(ps, ch, j=j, jw=jw):
                        sl = slice(ch * 512, (ch + 1) * 512)
                        tmp = tpool.tile([128, 512], bf16, tag="evt")
                        nc.vector.tensor_tensor(tmp[:jw, :], ps[:jw, :], x0[:jw, j, sl], MULT)
                        nc.vector.tensor_tensor(xdst[:jw, j, sl], tmp[:jw, :], xsrc[:jw, j, sl], ADD)

                    dense_layer(Wc_d[i], bc_d[i:i + 1], xsrc, j, jw, evict)

            def mlp_w0(x0, ha, ha8):
                for j in range(MT):
                    def evict(ps, ch, j=j):
                        sl = slice(ch * 512, (ch + 1) * 512)
                        nc.scalar.activation(ha[j][:, sl], ps[:, :], RELU)
                        nc.vector.tensor_copy(ha8[:, j, sl], ha[j][:, sl])
                    dense_layer(W0_d[:], b0_d[:], x0, j, 128, evict)

            def mlp_hidden(l, src8, dst, dst8):
                for j in range(MT):
                    j0 = j * 128
                    whm = wpool.tile([128, MT, 128], f8, tag="whid")
                    nc.sync.dma_start(
                        whm[:, :, :], Wh_d[l, :, j0:j0 + 128]
                        .rearrange("(ko q) n -> q ko n", q=128))
                    bias = bpool.tile([128, 1], f32, tag="bias")
                    nc.sync.dma_start(bias[:], bhT_d[j0:j0 + 128, l:l + 1])
                    pss = [mmps.tile([128, 512], mybir.dt.float32, space="PSUM",
                                     name="psacc", tag="psacc") for _ in range(NCH)]
                    for t in range(MT // 2):
                        for ch in range(NCH):
                            nc.tensor.matmul(
                                pss[ch][:, :], whm[:, 2 * t:2 * t + 2, :],
                                src8[:, 2 * t:2 * t + 2, ch * 512:(ch + 1) * 512],
                                start=(t == 0), stop=(t == MT // 2 - 1),
                                perf_mode=DR)
                    for ch in range(NCH):
                        sl = slice(ch * 512, (ch + 1) * 512)
                        nc.scalar.activation(dst[j][:, sl], pss[ch][:, :],
                                             RELU, bias=bias[:])
                        if dst8 is not None:
                            nc.vector.tensor_copy(dst8[:, j, sl], dst[j][:, sl])

            def final_layer(p, xfin, hfin):
                wfm = wpool.tile([128, 20, 1], f8, tag="wfm")
                nc.sync.dma_start(
                    wfm[:], Wfx_d[0:2560, 0:1].rearrange("(ko q) n -> q ko n", q=128))
                wfl = wpool.tile([128, 1], f8, tag="wfl")
                nc.sync.dma_start(wfl[0:64, :], Wfx_d[2560:2624, 0:1])
                nc.sync.dma_start(wfl[64:65, :], bf_d[:])
                wfh = wpool.tile([128, MT, 1], bf16, tag="wfh")
                nc.sync.dma_start(
                    wfh[:], Wfh_d[:, 0:1].rearrange("(ko q) n -> q ko n", q=128))
                for ch in range(NCH):
                    sl = slice(ch * 512, (ch + 1) * 512)
                    zps = mmps.tile([128, 512], mybir.dt.float32, space="PSUM", tag="psacc")
                    for t in range(20):
                        nc.tensor.matmul(zps[0:1, :], wfm[:, t, :], xfin[:, t, sl],
                                         start=(t == 0), stop=False)
                    nc.tensor.matmul(zps[0:1, :], wfl[0:65, :], xfin[0:65, 20, sl],
                                     start=False, stop=False)
                    for t in range(MT):
                        nc.tensor.matmul(zps[0:1, :], wfh[:, t, :],
                                         hfin[t][:, sl],
                                         start=False, stop=(t == MT - 1))
                    zsb = zpool.tile([1, 512], f32, tag="zsb")
                    nc.scalar.activation(zsb[:], zps[0:1, :], SIGM)
                    nc.sync.dma_start(
                        out_d[0:1, p * BC + ch * 512: p * BC + (ch + 1) * 512], zsb[:])

            for pi, p in enumerate([pp for _ in range(repeats) for pp in range(N_PASS)]):
                # Pass p's x0 gets its own family (0 or 3) so pass p+1's
                # gathers start immediately instead of WAR-waiting on the
                # cross ping-pong buffers; ping-pong uses families 1/2.
                x0fam = 0 if pi % 2 == 0 else 3
                afam = 1
                x0 = alloc_x(f"xs{x0fam}_")
                assemble_x0(p, x0)
                if debug_x0:
                    for t in range(KT):
                        dbg = tpool.tile([128, 512], f32, tag="dbg")
                        for ch in range(NCH):
                            nc.vector.tensor_copy(dbg[:], x0[:, t, ch * 512:(ch + 1) * 512])
                            nc.sync.dma_start(
                                x0dbg_d[p, t, :, ch * 512:(ch + 1) * 512], dbg[:])
                bufs = [alloc_x(f"xs{afam}_"), alloc_x("xs2_")]
                for bb_ in bufs:
                    nc.vector.memset(bb_[64:65, 20, :], 1.0)
                # Interleave the MLP with the cross layers (they only meet at
                # the final concat): W0 after cross 0, hidden l after cross
                # l+1.  Each MLP layer's evict+cast barrier then hides under
                # the next cross layer's matmul stream instead of stacking
                # into one dependency-bound MLP region.
                interleave = with_mlp and n_cross == L_CROSS and not debug_x
                if with_mlp:
                    ha = [xpool.tile([128, BC], bf16, tag=f"ha{t}", name=f"ha{t}") for t in range(MT)]
                    hb = [xpool.tile([128, BC], bf16, tag=f"hb{t}", name=f"hb{t}") for t in range(MT)]
                    ha8 = xpool.tile([128, MT, BC], f8, tag="ha8", name="ha8")
                    hb8 = xpool.tile([128, MT, BC], f8, tag="hb8", name="hb8")

                hsrc, hsrc8 = None, None

                def mlp_step(s):
                    nonlocal hsrc, hsrc8
                    if s == 0:
                        mlp_w0(x0, ha, ha8)
                        hsrc, hsrc8 = ha, ha8
                    else:
                        l = s - 1
                        hdst = hb if l % 2 == 0 else ha
                        hdst8 = hb8 if l % 2 == 0 else ha8
                        last = l == N_MLP_HID - 1
                        mlp_hidden(l, hsrc8, hdst, None if last else hdst8)
                        hsrc, hsrc8 = hdst, hdst8

                xsrc = x0
                for i in range(n_cross):
                    xdst = bufs[i % 2]
                    cross_layer(i, x0, xsrc, xdst)
                    xsrc = xdst
                    if interleave:
                        mlp_step(i)
                xfin = xsrc
                if debug_x:
                    for t in range(KT):
                        dbg = tpool.tile([128, 512], f32, tag="dbg")
                        for ch in range(NCH):
                            nc.vector.tensor_copy(dbg[:], xfin[:, t, ch * 512:(ch + 1) * 512])
                            nc.sync.dma_start(
                                xdbg_d[p, t, :, ch * 512:(ch + 1) * 512], dbg[:])
                if with_mlp:
                    if not interleave:
                        for s in range(1 + N_MLP_HID):
                            mlp_step(s)
                    final_layer(p, xfin, hsrc)

    # Route each SWDGE DMA to the hardware queue matching its DMASW lane.
    # Tile's sem assignment distributes SWDGE DMAs round-robin over 8 DMASW
    # lanes (one vector-clock dim + sem per lane) and assumes completions
    # within a lane are FIFO.  Mapping queue = lane % 4 keeps every lane on
    # a single hardware queue, so in-lane FIFO still holds while the 4
    # queues run descriptor generation in parallel.
    if nc.num_swdge_queues > 1:
        from concourse.tile_scheduler import PROC_NAME_TO_IDX
        sw0 = PROC_NAME_TO_IDX["DMASW0"]
        sw7 = PROC_NAME_TO_IDX["DMASW7"]
        for inst in swdge_insts:
            if hasattr(inst, "ins"):
                inst = inst.ins
            proc = inst.bass_scheduled_proc
            assert proc is not None and sw0 <= proc <= sw7, (inst.name, proc)
            q = (proc - sw0) % nc.num_swdge_queues
            if isinstance(inst, mybir.InstDMAGatherAnt):
                inst.queue_num = q
            elif isinstance(inst, mybir.InstDMACopy):
                inst.queue = f"qPoolDynamic{q if q else ''}"
            else:
                raise AssertionError(f"unexpected SWDGE inst {type(inst)}")

    nc.compile()
    return nc


# needed at module level for the builder
import concourse.bass as bass  # noqa: E402


def _prep_core_inputs(core, user_input, item_input, numeric_feats, categorical_feats,
                      shared):
    r0 = core * B_CORE
    u = user_input[r0:r0 + B_CORE]
    it = item_input[r0:r0 + B_CORE]
    num = numeric_feats[r0:r0 + B_CORE]
    cat = categorical_feats[r0:r0 + B_CORE]

    u_idx = np.ascontiguousarray(u.reshape(16, 128).T).astype(np.int32)
    i_idx = np.ascontiguousarray(it.reshape(16, 128).T).astype(np.int32)

    c_idx = np.zeros((128, N_CAT * 128), np.int16)
    for f in range(N_CAT):
        for p in range(N_PASS):
            seg = cat[p * BC:(p + 1) * BC, f].astype(np.int16)
            blk = seg.reshape(BC // 16, 16).T          # wrap-A: idx i at [i%16, i//16]
            c_idx[:, f * 128 + p * 64: f * 128 + (p + 1) * 64] = np.tile(blk, (8, 1))

    import ml_dtypes
    numT = np.empty((N_NUM + 1, B_CORE), ml_dtypes.bfloat16)
    numT[:N_NUM] = num.T.astype(ml_dtypes.bfloat16)
    numT[N_NUM] = 1.0

    return {
        "u_idx": u_idx, "i_idx": i_idx, "c_idx": c_idx, "numT": numT,
        **shared,
    }


def _prep_shared(num_W, num_b, user_emb, item_emb, cat_tables,
                 Wc, bc, W0, b0, Wh, bh, Wf, bf):
    import ml_dtypes
    ndiag = np.zeros((N_NUM + 1, N_NUM * EMB), np.float32)
    for f in range(N_NUM):
        ndiag[f, f * EMB:(f + 1) * EMB] = num_W[f]
    ndiag[N_NUM] = num_b.reshape(-1)

    cat_pad = np.zeros((N_CAT * CAT_VOCAB, 128), ml_dtypes.bfloat16)
    ct = cat_tables.astype(ml_dtypes.bfloat16)
    for f in range(N_CAT):
        sl = slice(f * CAT_VOCAB, (f + 1) * CAT_VOCAB)
        if f % 2 == 0:   # destination rows 64:128 of the x^T tile
            cat_pad[sl, 64:128] = ct[f]
        else:            # destination rows 0:64
            cat_pad[sl, 0:64] = ct[f]

    bf16 = ml_dtypes.bfloat16
    fp8 = ml_dtypes.float8_e4m3
    return {
        "ndiag": ndiag.astype(bf16),
        "user_emb": np.ascontiguousarray(user_emb, np.float32),
        "item_emb": np.ascontiguousarray(item_emb, np.float32),
        "cat_pad": cat_pad,
        "Wc": np.ascontiguousarray(Wc, np.float32).astype(fp8),
        "bcx": np.ascontiguousarray(bc, np.float32).astype(fp8),
        "W0": np.ascontiguousarray(W0, np.float32).astype(fp8),
        "b0": np.ascontiguousarray(b0, np.float32).reshape(1, MLP).astype(fp8),
        "Wh": np.ascontiguousarray(Wh, np.float32).astype(fp8),
        "bhT": np.ascontiguousarray(np.asarray(bh, np.float32).T),
        "Wfx": np.ascontiguousarray(Wf[:D], np.float32).astype(fp8),
        "Wfh": np.ascontiguousarray(Wf[D:], np.float32).astype(bf16),
        "bf": np.asarray(bf, np.float32).reshape(1, 1).astype(fp8),
    }


def make_in_maps(user_input, item_input, numeric_feats, categorical_feats,
                 user_emb, item_emb, cat_tables, num_W, num_b,
                 Wc, bc, W0, b0, Wh, bh, Wf, bf):
    user_input = np.asarray(user_input).astype(np.int64)
    item_input = np.asarray(item_input).astype(np.int64)
    numeric_feats = np.asarray(numeric_feats, np.float32)
    categorical_feats = np.asarray(categorical_feats).astype(np.int64)
    shared = _prep_shared(np.asarray(num_W, np.float32), np.asarray(num_b, np.float32),
                          np.asarray(user_emb), np.asarray(item_emb),
                          np.asarray(cat_tables, np.float32),
                          np.asarray(Wc), np.asarray(bc), np.asarray(W0),
                          np.asarray(b0), np.asarray(Wh), np.asarray(bh),
                          np.asarray(Wf), np.asarray(bf))
    return [
        _prep_core_inputs(core, user_input, item_input, numeric_feats,
                          categorical_feats, shared)
        for core in range(CORES)
    ]


def get_nc(**flags):
    key = tuple(sorted(flags.items()))
    if key not in _CACHE:
        _CACHE[key] = _build_nc(**flags)
    return _CACHE[key]


def kernel(**inputs) -> np.ndarray:
    from concourse.bass_utils import run_bass_kernel_spmd
    nc = get_nc()
    in_maps = make_in_maps(**inputs)
    res = run_bass_kernel_spmd(nc, in_maps, list(range(CORES)))
    out = np.concatenate([res.results[i]["out"][0] for i in range(CORES)])
    return out.reshape(B, 1).astype(np.float32)

